# revision 91
# baseline (speedup 1.0000x reference)
"""Trainium2 Bass kernel for nn_AttnDecoderRNN (B=32,T=20,L=49,F=512,H=1024,V=32000).

Zero-collective design across 8 NeuronCores:
- The attention-LSTM recurrence is fully REPLICATED on every core (tensor-
  parallel splits need a per-step AllGather of h, which dominates cost).
- Only the vocab projection is tensor-parallel: core k owns W_out rows
  [4000k, 4000(k+1)) and emits a (640, 4000) bf16 logit shard; the host
  concatenation is the all-gather. b_out is added host-side (skipped if 0).

Per-core schedule (all layouts feature-on-partition):
- scoresT attention: sps[(l,b), b'] = feats . (Wa^T h), diagonal extracted
  with a one-hot mask + grouped reduce; softmax runs in the (l%4 x b, l//4)
  layout with tiny PE sum/broadcast matmuls; normalize+expand fused into one
  scalar_tensor_tensor.
- gates: one PSUM accumulation (bias matmul + iw + h + ctx parts); the
  LSTM elementwise runs mostly in place on PSUM; a dummy sigmoid after the
  softmax Exp prefetches the ACT table set off the critical chain.
- dec groups (4 steps) with Wh2o/Wc2o streamed from HBM; vocab chunks are
  interleaved into the step loop (W_out streamed) to keep the PE fed
  through the softmax/LSTM dependency-chain windows.
"""
import sys

sys.path.insert(0, "/opt/trn_rl_repo")
import numpy as np
import ml_dtypes

import concourse.bass as bass
import concourse.mybir as mybir
import concourse.tile as tile
from concourse import bacc
from concourse.bass_utils import run_bass_kernel_spmd

B, T, L, F, H, V = 32, 20, 49, 512, 1024, 32000
LP = 64
MB = 13            # l-blocks kept (l < 4*MB covers L=49; blocks 13-15 are all-pad)
NC = 8
VS = V // NC      # 4000
CW = 250          # vocab chunk width (16 chunks per group)
BF = mybir.dt.bfloat16
F32 = mybir.dt.float32
NBF = ml_dtypes.bfloat16
NF8 = ml_dtypes.float8_e3m4
NE4 = ml_dtypes.float8_e4m3
WSCALE = 512.0         # gate weight pre-scale (e4m3 normal range)
XS = 32.0              # gate rhs (h/ctx/iw) pre-scale for e4m3
GPERM = [2, 1, 0, 3]   # gate storage g,f,i,o: bank0={g,f} early, bank1={i,o} late
HEAT_A = 0            # heater matmuls filling the softmax window
HEAT_B = 0            # heater matmuls filling the LSTM-tail window
RSKIP = 6              # steps running without the Whh Q2 residual (DMA window)
# DoubleRow fp8 gates: weights pair-quantized (Q1+Q2) where listed 3,
# single-quantized (Q1 only) where 2; rhs always pair-quantized. Each
# DR matmul covers 2 k-tiles at 0.5 cycles/row. Wi1 (value 0) stays
# plain e3m4 at x128 with a bf16 ctx rhs scaled x128 (rbBs carries the
# 128; Wc2o is pre-divided by 128 to compensate on the dec path) --
# its pair-residual would not fit SBUF.
PLAN = {"Whh": 3, "Wi1": 0, "Wi2": 2}


def _qpair(x):
    q1 = x.astype(NE4)
    q2 = (x - q1.astype(np.float32)).astype(NE4)
    return q1, q2

_BUILT = {}
SECTIONS = []


def _sec(nc, label):
    SECTIONS.append((int(nc.get_next_instruction_name().split('-')[1]), label))


def host_prep(inputs):
    f32 = lambda x: np.asarray(x, np.float32)
    feats = f32(inputs["features"])                    # (B, F, L)
    cap = np.asarray(inputs["captions"])
    emb = np.asarray(inputs["embed_table"])
    fpad = np.zeros((LP, B, F), np.float32)
    fpad[:L] = feats.transpose(2, 0, 1)
    # 8-wide score layout: partition p8 = (l%16)*8 + b%8, col blocks (m=l//16,
    # g=b//8); featsT8 (512, [kt is rows][m][g][p8])
    featsT = np.ascontiguousarray(
        fpad.reshape(4, 16, 4, 8, 512).transpose(4, 0, 2, 1, 3)
        .reshape(512, 2048))
    h0 = np.tanh(feats.mean(axis=2) @ f32(inputs["W_init"]).T + f32(inputs["b_init"]))
    h0T = h0.T                                         # (1024, 32)
    h0slot = np.ascontiguousarray(
        h0T.reshape(8, 128, B).transpose(1, 0, 2)).reshape(128, 256)
    h0q1, h0q2 = _qpair(h0slot.astype(NBF).astype(np.float32) * XS)
    e = f32(emb[cap])
    iw = np.concatenate([np.zeros((B, 1, F), np.float32), e[:, :-1]], axis=1)
    iwT = np.ascontiguousarray(iw.transpose(2, 1, 0)).reshape(F, T * B)
    iwq1, iwq2 = _qpair(iwT.astype(NBF).astype(np.float32) * XS)
    Wih = f32(inputs["W_ih"])
    # permute gate blocks: storage pos p holds original gate GPERM[p]
    def gperm_cols(W):                                 # W (K, 4096) -> permuted
        return np.ascontiguousarray(
            W.reshape(-1, 4, 1024)[:, GPERM].reshape(-1, 4096))
    Whh = gperm_cols(f32(inputs["W_hh"]).T)            # (1024, 4096)
    Wi1 = gperm_cols(Wih[:, :F].T)                     # (512, 4096)
    Wi2 = gperm_cols(Wih[:, F:].T)                     # (512, 4096)
    bg = (f32(inputs["b_ih"]) + f32(inputs["b_hh"])).reshape(4, 8, 128)[GPERM]
    indic32 = np.zeros((32, 1024), np.float32)
    for k in range(32):
        indic32[k, k * 32:(k + 1) * 32] = 1.0
    # maskE8 cols (g, m, c8): diag selector c == p8%8
    maskE = np.zeros((128, 128), np.float32)
    for p in range(128):
        for g in range(4):
            for m in range(4):
                maskE[p, g * 32 + m * 8 + (p % 8)] = 1.0
    # padT8 cols (g, m): pad where l = 16m + p8//8 >= L
    padT = np.zeros((128, 16), np.float32)
    for p in range(128):
        for g in range(4):
            for m in range(4):
                if 16 * m + p // 8 >= L:
                    padT[p, g * 4 + m] = -30000.0
    Pg = np.zeros((128, 8), np.float32)
    for p in range(128):
        Pg[p, p % 8] = 1.0
    bdec = (f32(inputs["b_h2o"]) + f32(inputs["b_c2o"])).reshape(4, 128).T
    shared = {
        "featsT": featsT.astype(NBF),
        "h0slot": h0slot.astype(NBF), "c0": h0slot.astype(np.float32),
        "h0q1": h0q1, "h0q2": h0q2,
        "iwq1": iwq1, "iwq2": iwq2,
        "ones32": np.ones((8, 128), np.float32),
        "Wa": f32(inputs["Wa"]).astype(NBF),           # (1024, 512) lhsT
        # gate weights pair-quantized e4m3 at xWSCALE (DoubleRow matmuls);
        # rhs x-side pair-quantized at xXS; activation scale 1/(WSCALE*XS)
        "biasLhs": (bg.reshape(32, 128) * WSCALE * XS).astype(NBF),
        "indic32": indic32.astype(NBF),
        "Wh2o": f32(inputs["W_h2o"]).T.astype(NBF),    # (1024, 512)
        # ctxh carries ctx*128 (for the e3m4 Wi1 path); Wc2o compensates
        "Wc2o": (f32(inputs["W_c2o"]).T / 128.0).astype(NBF),  # (512, 512)
        "bdec": np.ascontiguousarray(bdec),
        "maskE": maskE.astype(NBF), "padT": padT,
        "Pg": Pg,
        "ident": np.eye(128, dtype=np.float32).astype(NBF),
        "identS": (np.eye(128, dtype=np.float32) / XS).astype(NBF),
    }
    for nm, W in (("Whh", Whh), ("Wi1", Wi1), ("Wi2", Wi2)):
        if PLAN[nm] == 0:
            shared[nm] = (W * 128.0).astype(NF8)
            continue
        q1, q2 = _qpair(W * WSCALE)
        shared[nm] = q1
        if PLAN[nm] == 3:
            shared[nm + "R"] = q2
    WoutT = f32(inputs["W_out"]).T                     # (512, 32000)
    in_maps = []
    for k in range(NC):
        m = dict(shared)
        # partition-major chunk layout: row p holds [ci][kt][n] so each
        # chunk DMA is one contiguous (128, 1000-elem) slice (>=512B runs)
        m["Wout"] = np.ascontiguousarray(
            WoutT[:, VS * k:VS * (k + 1)].reshape(4, 128, 16, CW)
            .transpose(1, 2, 0, 3).reshape(128, 64 * CW)).astype(NBF)
        in_maps.append(m)
    return in_maps


def _load_tiled(nc, pool, dram, KT, N, dtype, name):
    """dram (KT*128, N) -> sbuf (128, KT*N), col block kt holds rows kt*128.."""
    t = pool.tile([128, KT * N], dtype, name=name)
    src = dram[:].rearrange("(r p) n -> p r n", p=128)
    dst = t[:].rearrange("p (r n) -> p r n", n=N)
    nc.sync.dma_start(dst, src)
    return t


def build(repeat=1, dbg=False):
    nc = bacc.Bacc("TRN2", target_bir_lowering=False, debug=False, num_devices=NC)
    di = lambda nm, sh, dt: nc.dram_tensor(nm, list(sh), dt, kind="ExternalInput")
    featsT_d = di("featsT", (512, 2048), BF)
    ones32_d = di("ones32", (8, 128), F32)
    h0_d = di("h0slot", (128, 256), BF)
    c0_d = di("c0", (128, 256), F32)
    F8E4 = mybir.dt.float8e4
    h0q1_d = di("h0q1", (128, 256), F8E4)
    h0q2_d = di("h0q2", (128, 256), F8E4)
    iwq1_d = di("iwq1", (512, 640), F8E4)
    iwq2_d = di("iwq2", (512, 640), F8E4)
    Wa_d = di("Wa", (1024, 512), BF)
    F8 = mybir.dt.float8e3
    Whh_d = di("Whh", (1024, 4096), F8E4)
    Wi1_d = di("Wi1", (512, 4096), F8)
    Wi2_d = di("Wi2", (512, 4096), F8E4)
    WhhR_d = di("WhhR", (1024, 4096), F8E4)
    biasLhs_d = di("biasLhs", (32, 128), BF)
    indic32_d = di("indic32", (32, 1024), BF)
    Wh2o_d = di("Wh2o", (1024, 512), BF)
    Wc2o_d = di("Wc2o", (512, 512), BF)
    bdec_d = di("bdec", (128, 4), F32)
    Wout_d = di("Wout", (128, 64 * CW), BF)
    maskE_d = di("maskE", (128, 128), BF)
    padT_d = di("padT", (128, 16), F32)
    Pg_d = di("Pg", (128, 8), F32)

    ident_d = di("ident", (128, 128), BF)
    identS_d = di("identS", (128, 128), BF)
    out_d = nc.dram_tensor("out", [T * B, VS], BF, kind="ExternalOutput")
    if dbg:
        dbg_hist = nc.dram_tensor("dbg_hist", [128, 8 * 256], BF, kind="ExternalOutput")
        dbg_ctxh = nc.dram_tensor("dbg_ctxh", [128, 8 * 128], BF, kind="ExternalOutput")
        dbg_cT = nc.dram_tensor("dbg_cT", [128, 256], F32, kind="ExternalOutput")

    AF = mybir.ActivationFunctionType
    AX = mybir.AxisListType
    ALU = mybir.AluOpType
    with tile.TileContext(nc) as tc:
        with tc.tile_pool(name="cst", bufs=1) as cst, \
             tc.tile_pool(name="wk", bufs=3) as wk, \
             tc.tile_pool(name="wkd", bufs=3) as wkd, \
             tc.tile_pool(name="wkv", bufs=2) as wkv, \
             tc.tile_pool(name="psg", bufs=2, space="PSUM") as psg, \
             tc.tile_pool(name="psv", bufs=2, space="PSUM") as psv, \
             tc.tile_pool(name="psps", bufs=1, space="PSUM") as psps, \
             tc.tile_pool(name="psm", bufs=1, space="PSUM") as psm:
            # ---- persistent SBUF ----
            hist = cst.tile([128, 8 * 256], BF, name="hist")
            cT = cst.tile([128, 256], F32, name="cT")
            # Wa first (u needs it + h0; the half-split lets the r-phased u
            # accumulation start on half 1), then the tiny h0/c0/hq seeds
            Wa = cst.tile([128, 8 * 512], BF, name="Wa")
            for h_ in range(2):
                nc.sync.dma_start(
                    Wa[:, h_ * 4 * 512:(h_ + 1) * 4 * 512]
                    .rearrange("p (r n) -> p r n", n=512),
                    Wa_d[h_ * 512:(h_ + 1) * 512, :]
                    .rearrange("(r p) n -> p r n", p=128))
            nc.sync.dma_start(hist[:, 7 * 256:8 * 256], h0_d[:])
            nc.sync.dma_start(cT[:], c0_d[:])
            # per-step e4m3 pair quantizations of h (persistent, rewritten
            # each step; seeded with h0 pairs -- must land before step-0 gates)
            hq1 = cst.tile([128, 256], F8E4, name="hq1")
            hq2 = cst.tile([128, 256], F8E4, name="hq2")
            nc.sync.dma_start(hq1[:], h0q1_d[:])
            nc.sync.dma_start(hq2[:], h0q2_d[:])
            featsT = cst.tile([128, 4 * 2048], BF, name="featsT")
            for kt_ in range(4):
                nc.sync.dma_start(
                    featsT[:, kt_ * 2048:(kt_ + 1) * 2048],
                    featsT_d[kt_ * 128:(kt_ + 1) * 128, :])
            small = [("biasLhs", biasLhs_d, [32, 128], BF),
                     ("indic32", indic32_d, [32, 1024], BF),
                     ("bdec", bdec_d, [128, 4], F32),
                     ("maskE", maskE_d, [128, 128], BF),
                     ("padT", padT_d, [128, 16], F32),
                     ("Pg", Pg_d, [128, 8], F32),
                     ("ones32", ones32_d, [8, 128], F32),
                     ("ident", ident_d, [128, 128], BF),
                     ("identS", identS_d, [128, 128], BF)]
            sm = {}
            for nm, d, sh, dt in small:
                sm[nm] = cst.tile(sh, dt, name=nm)
                nc.sync.dma_start(sm[nm][:], d[:])
            # Whh split into quarters: each k-pair's gates start as its
            # 1MB chunk lands at startup
            Whh = cst.tile([128, 8 * 4096], F8E4, name="Whh")
            for h_ in range(4):
                nc.sync.dma_start(
                    Whh[:, h_ * 2 * 4096:(h_ + 1) * 2 * 4096]
                    .rearrange("p (r n) -> p r n", n=4096),
                    Whh_d[h_ * 256:(h_ + 1) * 256, :]
                    .rearrange("(r p) n -> p r n", p=128))
            # fblk derived on-chip from featsT (PE transposes + DVE copies)
            # instead of a second 2MB DMA of the same data
            fblk = cst.tile([128, 16 * 512], BF, name="fblk")
            # Wi1 split into kt-halves: step-0 wi1 phase 0 starts ~3us sooner
            Wi1 = cst.tile([128, 4 * 4096], F8, name="Wi1")
            for h_ in range(2):
                nc.sync.dma_start(
                    Wi1[:, h_ * 2 * 4096:(h_ + 1) * 2 * 4096]
                    .rearrange("p (r n) -> p r n", n=4096),
                    Wi1_d[h_ * 256:(h_ + 1) * 256, :]
                    .rearrange("(r p) n -> p r n", p=128))
            # Wi2/iwq needed first at t=1 (iw_0 is zeros): load after Wi1
            iwq1 = _load_tiled(nc, cst, iwq1_d, 4, 640, F8E4, "iwq1")
            iwq2 = _load_tiled(nc, cst, iwq2_d, 4, 640, F8E4, "iwq2")
            # Wi2 in halves: t=1's gates_iw pair 0 starts on half 1
            Wi2 = cst.tile([128, 4 * 4096], F8E4, name="Wi2")
            for h_ in range(2):
                nc.sync.dma_start(
                    Wi2[:, h_ * 2 * 4096:(h_ + 1) * 2 * 4096]
                    .rearrange("p (r n) -> p r n", n=4096),
                    Wi2_d[h_ * 256:(h_ + 1) * 256, :]
                    .rearrange("(r p) n -> p r n", p=128))
            Wh2o = _load_tiled(nc, cst, Wh2o_d, 8, 512, BF, "Wh2o")
            Wc2o = _load_tiled(nc, cst, Wc2o_d, 4, 512, BF, "Wc2o")
            WoutS = cst.tile([128, 64 * CW], BF, name="WoutS")
            nc.sync.dma_start(WoutS[:], Wout_d[:])
            # Whh residual (Q2) loads last; steps t < RSKIP run Q1-only
            WhhR = _load_tiled(nc, cst, WhhR_d, 8, 4096, F8E4, "WhhR")
            biasLhs, indic32, bdec = sm["biasLhs"], sm["indic32"], sm["bdec"]
            maskE, padT, Pg, ones32, ident, identS = (
                sm["maskE"], sm["padT"], sm["Pg"], sm["ones32"], sm["ident"],
                sm["identS"])
            ctxh = cst.tile([128, 8 * 128], BF, name="ctxh")
            # decT ring of 2 groups: col kt*256 + (g%2)*128 + (t%4)*32 + b
            decT = cst.tile([128, 4 * 256], BF, name="decT")
            mm = nc.tensor.matmul

            def gblk(gp, bi):
                # gates stored in two bank-sized tiles (separate dependency
                # tracking): A = {g,f} blocks bi 0-15, B = {i,o} blocks 16-31
                if bi < 16:
                    return gp[0][:, bi * 32:(bi + 1) * 32]
                return gp[1][:, (bi - 16) * 32:(bi - 15) * 32]

            DRM = mybir.MatmulPerfMode.DoubleRow
            Wi2v = Wi2[:].rearrange("p (pr two n) -> p pr two n", pr=2, two=2)
            Whhv = Whh[:].rearrange("p (pr two n) -> p pr two n", pr=4, two=2)
            WhhRv = WhhR[:].rearrange("p (pr two n) -> p pr two n", pr=4, two=2)
            iwq1v = iwq1[:].rearrange("p (kt n) -> p kt n", kt=4)
            iwq2v = iwq2[:].rearrange("p (kt n) -> p kt n", kt=4)
            hq1v = hq1[:].rearrange("p (pr two n) -> p pr two n", pr=4, two=2)
            hq2v = hq2[:].rearrange("p (pr two n) -> p pr two n", pr=4, two=2)

            def gates_iw(t):
                _sec(nc, 'gatesiw')
                """allocate gps pair for step t; bias + iw part (no h/ctx dep)"""
                gp = (psg.tile([128, 512], F32, name="gpsA", tag="ga"),
                      psg.tile([128, 512], F32, name="gpsB", tag="gb"))
                mm(gp[0][:], biasLhs[:], indic32[:, 0:512],
                   start=True, stop=False)
                mm(gp[1][:], biasLhs[:], indic32[:, 512:1024],
                   start=True, stop=False)
                if t == 0:
                    # iw_0 is exact zeros by construction: its matmuls are an
                    # identity no-op; skipping them lets Wi2/iwq load after
                    # Wi1 (off the startup critical path)
                    return gp
                for pr in range(2):
                    for bi in range(32):
                        blk = gblk(gp, bi)
                        w1 = Wi2v[:, pr, :, bi * 128:(bi + 1) * 128]
                        for xv in (iwq1v, iwq2v):
                            mm(blk, w1,
                               xv[:, 2 * pr:2 * pr + 2, t * 32:(t + 1) * 32],
                               start=False, stop=False, perf_mode=DRM)
                return gp

            def dec_group(gq):
                _sec(nc, 'dec')
                # Wh2o/Wc2o lhsT tiles streamed from HBM per group
                s0 = (4 * gq) % 8
                g2 = gq % 2
                hv = hist[:].rearrange("p (s r b) -> p s r b", s=8, b=32)
                cv = ctxh[:].rearrange("p (s r b) -> p s r b", s=8, b=32)
                dps = psm.tile([128, 512], F32, name="dps", tag="x")
                for r in range(8):
                    for m in range(4):
                        mm(dps[:, m * 128:(m + 1) * 128],
                           Wh2o[:, r * 512 + m * 128: r * 512 + (m + 1) * 128],
                           hv[:, s0:s0 + 4, r, :],
                           start=(r == 0 and m == 0), stop=False)
                for r in range(4):
                    for m in range(4):
                        mm(dps[:, m * 128:(m + 1) * 128],
                           Wc2o[:, r * 512 + m * 128: r * 512 + (m + 1) * 128],
                           cv[:, s0:s0 + 4, r, :], start=False, stop=False)
                for m in range(4):
                    mm(dps[:, m * 128:(m + 1) * 128], identS[:],
                       iwq1[:, m * 640 + gq * 128: m * 640 + (gq + 1) * 128],
                       start=False, stop=False)
                for m in range(4):
                    mm(dps[:, m * 128:(m + 1) * 128], identS[:],
                       iwq2[:, m * 640 + gq * 128: m * 640 + (gq + 1) * 128],
                       start=False, stop=True)
                for m in range(4):
                    nc.scalar.activation(
                        decT[:, m * 256 + g2 * 128: m * 256 + (g2 + 1) * 128],
                        dps[:, m * 128:(m + 1) * 128], AF.Tanh,
                        bias=bdec[:, m:m + 1])

            def vocab_chunks(gq, cis, fine=False):
                # consecutive chunks paired into one output DMA so the store
                # has >=512B contiguous runs (sub-512B runs pay 2x latency);
                # fine=True (endgame drain) goes chunk-at-a-time so the
                # mm->copy->store pipeline is twice as deep
                _sec(nc, 'vocab')
                g2 = gq % 2
                cis = list(cis)
                i = 0
                while i < len(cis):
                    pair = cis[i:i + 1] if fine else cis[i:i + 2]
                    if len(pair) == 2 and pair[1] != pair[0] + 1:
                        pair = pair[:1]
                    lgv = wkv.tile([128, 2 * CW], BF, name="lgv")
                    vps = psv.tile([128, 2 * CW], F32, name="vps")
                    for j, ci in enumerate(pair):
                        for kt in range(4):
                            mm(vps[:, j * CW:(j + 1) * CW],
                               decT[:, kt * 256 + g2 * 128: kt * 256 + (g2 + 1) * 128],
                               WoutS[:, ci * 4 * CW + kt * CW: ci * 4 * CW + (kt + 1) * CW],
                               start=(kt == 0), stop=(kt == 3))
                    # GPSIMD can't read PSUM on HW; alternate ACT/DVE to
                    # spread the eviction load across both engines
                    if (pair[0] // 2) % 2 == 0:
                        nc.scalar.copy(lgv[:, 0:len(pair) * CW],
                                       vps[:, 0:len(pair) * CW])
                    else:
                        nc.vector.tensor_copy(lgv[:, 0:len(pair) * CW],
                                              vps[:, 0:len(pair) * CW])
                    nc.sync.dma_start(
                        out_d[gq * 128:(gq + 1) * 128,
                              pair[0] * CW: pair[0] * CW + len(pair) * CW],
                        lgv[:, 0:len(pair) * CW])
                    i += len(pair)

            for rep in range(repeat):
                if rep > 0:
                    nc.sync.dma_start(hist[:, 7 * 256:8 * 256], h0_d[:])
                    nc.sync.dma_start(cT[:], c0_d[:])
                gps = None
                for t in range(T):
                    s = (t - 1) % 8
                    w = t % 8
                    hs = lambda kt: hist[:, s * 256 + kt * 32: s * 256 + kt * 32 + 32]
                    # -- u = Wa^T h  (512, 32) as (128, 4x32)
                    _sec(nc, 'u')
                    pu = psm.tile([128, 128], F32, name="pu", tag="x")
                    for rh in range(2):
                        for m in range(4):
                            for r in range(rh * 4, rh * 4 + 4):
                                mm(pu[:, m * 32:(m + 1) * 32],
                                   Wa[:, r * 512 + m * 128: r * 512 + (m + 1) * 128],
                                   hs(r), start=(rh == 0 and m == 0 and r == 0),
                                   stop=(r == 7))
                    u = wk.tile([128, 128], BF, name="u", bufs=2)
                    nc.scalar.copy(u[:], pu[:])
                    # -- scoresT (2048, 32) as (128, 16x32)
                    _sec(nc, 'scoresT')
                    sps = psps.tile([128, 128], F32, name="sps")
                    for kt in range(4):
                        for g in range(4):
                            for m in range(4):
                                mm(sps[:, g * 32 + m * 8: g * 32 + m * 8 + 8],
                                   featsT[:, kt * 2048 + (m * 4 + g) * 128:
                                          kt * 2048 + (m * 4 + g) * 128 + 128],
                                   u[:, kt * 32 + g * 8: kt * 32 + g * 8 + 8],
                                   start=(kt == 0 and g == 0 and m == 0),
                                   stop=(kt == 3))
                    if t == 0:
                        # derive fblk = featsT^T block-wise (PE transpose via
                        # identity rhs, Pool copies PSUM->SBUF); runs in the
                        # startup DMA window, saves a 2MB HBM load
                        _sec(nc, 'transp')
                        for mg in range(16):
                            tps = psv.tile([128, 4 * 128], BF, name="tps",
                                           tag="vps")
                            for kt in range(4):
                                mm(tps[:, kt * 128:(kt + 1) * 128],
                                   featsT[:, kt * 2048 + mg * 128:
                                          kt * 2048 + mg * 128 + 128],
                                   ident[:], is_transpose=True,
                                   start=True, stop=True)
                            nc.vector.tensor_copy(
                                fblk[:, mg * 512:(mg + 1) * 512], tps[:])
                    # -- gates bias+iw part (t=0 only; later steps emit it
                    # at the end of the previous step to fill the LSTM gap)
                    _sec(nc, 'gates0')
                    if gps is None:
                        gps = gates_iw(0)
                    _sec(nc, 'gatesWhh')
                    # -- gates h part: DR 3-term (Q1w*x1, Q2w*x1, Q1w*x2),
                    # pair-phased so the first Whh DMA half serves step 0.
                    # Steps 0-1 skip the Q2w residual so its 4MB DMA can land
                    # off the startup critical path (err contribution tested
                    # in acc_model: negligible).
                    for prh in range(4):
                        for bi in range(32):
                            blk = gblk(gps, bi)
                            for pr in range(prh, prh + 1):
                                x1 = hq1v[:, pr, :, :]
                                mm(blk, Whhv[:, pr, :, bi * 128:(bi + 1) * 128],
                                   x1, start=False, stop=False, perf_mode=DRM)
                                if t >= RSKIP:
                                    mm(blk, WhhRv[:, pr, :, bi * 128:(bi + 1) * 128],
                                       x1, start=False, stop=False, perf_mode=DRM)
                                mm(blk, Whhv[:, pr, :, bi * 128:(bi + 1) * 128],
                                   hq2v[:, pr, :, :],
                                   start=False, stop=False, perf_mode=DRM)
                    _sec(nc, 'dec+v2')
                    # deferred dec for the previous group
                    if t % 4 == 0 and t > 0:
                        dec_group(t // 4 - 1)
                    # 2 vocab chunks fill the softmax window
                    if t >= 4:
                        _lo, _hi = [(0, 1), (1, 6), (6, 11), (11, 16)][t % 4]
                        _mid = _lo + (1 if t % 4 == 0 else 2)
                        vocab_chunks(t // 4 - 1, range(_lo, _mid))
                    # -- diag extract + softmax (no max-sub; |scores| < 88)
                    _sec(nc, 'softmax')
                    nc.vector.tensor_mul(sps[:], sps[:], maskE[:])
                    sd = wk.tile([128, 16], F32, name="sd", bufs=2)
                    nc.vector.reduce_sum(
                        sd[:], sps[:].rearrange("p (gm c) -> p gm c", c=8), axis=AX.X)
                    nc.vector.tensor_add(sd[:], sd[:], padT[:])
                    ex = wk.tile([128, 16], BF, name="ex", bufs=2)
                    nc.scalar.activation(ex[:], sd[:], AF.Exp)
                    rows = wk.tile([128, 4], F32, name="rows", bufs=2)
                    nc.vector.reduce_sum(
                        rows[:], ex[:].rearrange("p (g m) -> p g m", m=4),
                        axis=AX.X)
                    # dummy: pulls the sigmoid-set table load (1.3us) into the
                    # post-exp window instead of the LSTM critical chain
                    if t >= 4:
                        dum = wk.tile([128, 1], F32, name="dum")
                        nc.scalar.activation(dum[:], rows[:, 0:1], AF.Sigmoid)
                    if HEAT_A:
                        heat = psm.tile([128, 32], F32, name="heat", tag="x")
                        for _hk in range(HEAT_A):
                            mm(heat[:], Wa[:, 0:128], Wa[:, 0:32],
                               start=True, stop=True)
                    # unnormalized diag expansion FIRST: aEs = ex * maskE is
                    # the only gate for ctx; the 1/sum normalizer (pS/rS/rbB)
                    # is built while the ctx matmuls run and lands in rbBs for
                    # the eviction stt. pS/rbB live in the sps PSUM ring
                    # (dead after the mask-mul) to avoid a tag-x deadlock.
                    aEs = wk.tile([128, 128], BF, name="aEs", bufs=2)
                    nc.vector.scalar_tensor_tensor(
                        aEs[:].rearrange("p (gm c) -> p gm c", c=8),
                        ex[:].rearrange("p gm -> p gm ()").broadcast_to([128, 16, 8]),
                        1.0,
                        maskE[:].rearrange("p (gm c) -> p gm c", c=8),
                        ALU.mult, ALU.mult)
                    # -- ctxT (512, 32) as (128, 4x32)
                    _sec(nc, 'ctx')
                    cps = psm.tile([128, 128], F32, name="cps", tag="x")
                    for mf in range(4):
                        for g in range(4):
                            for ml in range(4):
                                mm(cps[:, mf * 32 + g * 8: mf * 32 + g * 8 + 8],
                                   fblk[:, (ml * 4 + g) * 512 + mf * 128:
                                        (ml * 4 + g) * 512 + mf * 128 + 128],
                                   aEs[:, g * 32 + ml * 8: g * 32 + ml * 8 + 8],
                                   start=(ml == 0), stop=(ml == 3))
                    # normalizer, hidden behind the ctx matmuls
                    _sec(nc, 'softmax')
                    pS = psps.tile([8, 4], F32, name="pS", tag="sps")
                    mm(pS[:], Pg[:], rows[:], start=True, stop=True)
                    rS = wk.tile([8, 4], F32, name="rS")
                    nc.vector.reciprocal(rS[:], pS[:])
                    # rSd cols (g, c): rS[p8, g] * (p8 == c), b-ordered
                    rSd = wk.tile([8, 32], F32, name="rSd")
                    nc.vector.scalar_tensor_tensor(
                        rSd[:].rearrange("p (g c) -> p g c", c=8),
                        rS[:].rearrange("p g -> p g ()").broadcast_to([8, 4, 8]),
                        128.0,
                        Pg[0:8, 0:8].rearrange("p c -> p () c").broadcast_to([8, 4, 8]),
                        ALU.mult, ALU.mult)
                    rbB = psps.tile([128, 32], F32, name="rbB", tag="sps")
                    mm(rbB[:], ones32[:], rSd[:], start=True, stop=True)
                    # HW: DVE may read only ONE non-scalar PSUM input; the
                    # norm stt also reads cps, so rbB must bounce via SBUF
                    rbBs = wk.tile([128, 32], F32, name="rbBs")
                    nc.vector.tensor_copy(rbBs[:], rbB[:])
                    _sec(nc, 'ctx')
                    # normalize while evicting: ctxh = cps * rbBs
                    nc.vector.scalar_tensor_tensor(
                        ctxh[:, w * 128:(w + 1) * 128]
                        .rearrange("p (k c) -> p k c", c=32),
                        cps[:].rearrange("p (k c) -> p k c", c=32), 1.0,
                        rbBs[:].rearrange("p c -> p () c").broadcast_to([128, 4, 32]),
                        ALU.mult, ALU.mult)
                    # -- gates ctx part: bank0 gates (g,f) first, their
                    # activations + c-mul overlap the bank1 (i,o) matmuls
                    # (bank-aligned so no PSUM-bank ping-pong)
                    _sec(nc, 'Wi1+act')
                    gG, gF, gI, gO = (gps[0][:, 0:256], gps[0][:, 256:512],
                                      gps[1][:, 0:256], gps[1][:, 256:512])
                    tGs = wk.tile([128, 256], BF, name="tGs", bufs=2)
                    def wi1_sec(gsec):
                        for bi in range(gsec * 8, gsec * 8 + 8):
                            blk = gblk(gps, bi)
                            for kt in range(4):
                                mm(blk, Wi1[:, kt * 4096 + bi * 128: kt * 4096 + (bi + 1) * 128],
                                   ctxh[:, w * 128 + kt * 32: w * 128 + (kt + 1) * 32],
                                   start=False, stop=(kt == 3))
                    wi1_sec(0)
                    wi1_sec(1)
                    wi1_sec(2)
                    wi1_sec(3)
                    gps_next = gates_iw(t + 1) if t + 1 < T else None
                    _sec(nc, 'Wi1+act')
                    nc.scalar.activation(tGs[:], gG, AF.Tanh, scale=1.0 / (WSCALE * XS))
                    sF = wk.tile([128, 256], BF, name="sF", bufs=2)
                    nc.scalar.activation(sF[:], gF, AF.Sigmoid, scale=1.0 / (WSCALE * XS))
                    nc.vector.tensor_mul(cT[:], cT[:], sF[:])
                    # gate activations land in SBUF bf16: frees the gps PSUM
                    # bank early and enables 2x DVE on the i*g / o*tanh muls
                    gIs = wk.tile([128, 256], BF, name="gIs", bufs=2)
                    nc.scalar.activation(gIs[:], gI, AF.Sigmoid, scale=1.0 / (WSCALE * XS))
                    gOs = wk.tile([128, 256], BF, name="gOs", bufs=2)
                    nc.scalar.activation(gOs[:], gO, AF.Sigmoid, scale=1.0 / (WSCALE * XS))
                    # -- LSTM-gap fillers: next step's dep-free gate matmuls
                    # first (no DMA dependency), then 2 more vocab chunks
                    _sec(nc, 'giw+v2b')
                    # -- p-state heater: dep-free dummy matmuls keep the PE
                    # clock streak alive through the LSTM tail (a broken
                    # streak costs ~3us of half-speed ramp-up)
                    if HEAT_B and t >= 15:
                        heat2 = psm.tile([128, 32], F32, name="heat", tag="x")
                        for _hk in range(HEAT_B):
                            mm(heat2[:], Wa[:, 0:128], Wa[:, 0:32],
                               start=True, stop=True)
                    # -- LSTM elementwise tail (activations emitted above)
                    _sec(nc, 'lstmtail')
                    nc.vector.tensor_mul(gIs[:], gIs[:], tGs[:])
                    nc.vector.tensor_add(cT[:], cT[:], gIs[:])
                    tCs = wk.tile([128, 256], BF, name="tCs", bufs=2)
                    nc.scalar.activation(tCs[:, 0:128], cT[:, 0:128], AF.Tanh)
                    nc.vector.tensor_mul(hist[:, w * 256: w * 256 + 128],
                                         gOs[:, 0:128], tCs[:, 0:128])
                    nc.scalar.activation(tCs[:, 128:256], cT[:, 128:256], AF.Tanh)
                    nc.vector.tensor_mul(hist[:, w * 256 + 128:(w + 1) * 256],
                                         gOs[:, 128:256], tCs[:, 128:256])
                    # e4m3 pair quantization of h for next step's DR gates
                    hsl = hist[:, w * 256:(w + 1) * 256]
                    nc.scalar.activation(hq1[:], hsl, AF.Copy, scale=XS)
                    nc.vector.scalar_tensor_tensor(
                        hq2[:], hsl, XS, hq1[:], ALU.mult, ALU.subtract)
                    # tail-slot vocab emitted AFTER the tail ops: their PE
                    # matmuls still fill this window, but the PSUM->SBUF
                    # copies now queue BEHIND the critical tail ACT/DVE ops
                    # instead of blocking them (in-order engine queues)
                    if t >= 4:
                        vocab_chunks(t // 4 - 1, range(_mid, _hi))
                    gps = gps_next
                # tail: last dec group + its vocab
                dec_group(4)
                vocab_chunks(4, range(16))
                if dbg:
                    nc.sync.dma_start(dbg_hist[:], hist[:])
                    nc.sync.dma_start(dbg_ctxh[:], ctxh[:])
                    nc.sync.dma_start(dbg_cT[:], cT[:])
    nc.finalize()
    return nc


def kernel(**inputs) -> np.ndarray:
    if "nc" not in _BUILT:
        _BUILT["nc"] = build()
    nc = _BUILT["nc"]
    in_maps = host_prep(inputs)
    res = run_bass_kernel_spmd(nc, in_maps, core_ids=list(range(NC)))
    full = np.concatenate(
        [np.asarray(res.results[k]["out"]) for k in range(NC)], axis=1)
    # (640, 32000) bf16, row t*32+b -> (B, T, V) f32
    out = np.ascontiguousarray(
        full.reshape(T, B, V).transpose(1, 0, 2)).astype(np.float32)
    b_out = np.asarray(inputs["b_out"], np.float32)
    if np.any(b_out):
        out += b_out[None, None, :]
    return out



# revision 92
# speedup vs baseline: 1.0147x; 1.0147x over previous
"""Trainium2 Bass kernel for nn_AttnDecoderRNN (B=32,T=20,L=49,F=512,H=1024,V=32000).

Zero-collective design across 8 NeuronCores:
- The attention-LSTM recurrence is fully REPLICATED on every core (tensor-
  parallel splits need a per-step AllGather of h, which dominates cost).
- Only the vocab projection is tensor-parallel: core k owns W_out rows
  [4000k, 4000(k+1)) and emits a (640, 4000) bf16 logit shard; the host
  concatenation is the all-gather. b_out is added host-side (skipped if 0).

Per-core schedule (all layouts feature-on-partition):
- scoresT attention: sps[(l,b), b'] = feats . (Wa^T h), diagonal extracted
  with a one-hot mask + grouped reduce; softmax runs in the (l%4 x b, l//4)
  layout with tiny PE sum/broadcast matmuls; normalize+expand fused into one
  scalar_tensor_tensor.
- gates: one PSUM accumulation (bias matmul + iw + h + ctx parts); the
  LSTM elementwise runs mostly in place on PSUM; a dummy sigmoid after the
  softmax Exp prefetches the ACT table set off the critical chain.
- dec groups (4 steps) with Wh2o/Wc2o streamed from HBM; vocab chunks are
  interleaved into the step loop (W_out streamed) to keep the PE fed
  through the softmax/LSTM dependency-chain windows.
"""
import sys

sys.path.insert(0, "/opt/trn_rl_repo")
import numpy as np
import ml_dtypes

import concourse.bass as bass
import concourse.mybir as mybir
import concourse.tile as tile
from concourse import bacc
from concourse.bass_utils import run_bass_kernel_spmd

B, T, L, F, H, V = 32, 20, 49, 512, 1024, 32000
LP = 64
MB = 13            # l-blocks kept (l < 4*MB covers L=49; blocks 13-15 are all-pad)
NC = 8
VS = V // NC      # 4000
CW = 250          # vocab chunk width (16 chunks per group)
BF = mybir.dt.bfloat16
F32 = mybir.dt.float32
NBF = ml_dtypes.bfloat16
NF8 = ml_dtypes.float8_e3m4
NE4 = ml_dtypes.float8_e4m3
WSCALE = 512.0         # gate weight pre-scale (e4m3 normal range)
XS = 32.0              # gate rhs (h/ctx/iw) pre-scale for e4m3
GPERM = [2, 1, 0, 3]   # gate storage g,f,i,o: bank0={g,f} early, bank1={i,o} late
HEAT_A = 0            # heater matmuls filling the softmax window
HEAT_B = 0            # heater matmuls filling the LSTM-tail window
RSKIP = 6              # steps running without the Whh Q2 residual (DMA window)
# DoubleRow fp8 gates: weights pair-quantized (Q1+Q2) where listed 3,
# single-quantized (Q1 only) where 2; rhs always pair-quantized. Each
# DR matmul covers 2 k-tiles at 0.5 cycles/row. Wi1 (value 0) stays
# plain e3m4 at x128 with a bf16 ctx rhs scaled x128 (rbBs carries the
# 128; Wc2o is pre-divided by 128 to compensate on the dec path) --
# its pair-residual would not fit SBUF.
PLAN = {"Whh": 3, "Wi1": 0, "Wi2": 2}


def _qpair(x):
    q1 = x.astype(NE4)
    q2 = (x - q1.astype(np.float32)).astype(NE4)
    return q1, q2

_BUILT = {}
SECTIONS = []


def _sec(nc, label):
    SECTIONS.append((int(nc.get_next_instruction_name().split('-')[1]), label))


def host_prep(inputs):
    f32 = lambda x: np.asarray(x, np.float32)
    feats = f32(inputs["features"])                    # (B, F, L)
    cap = np.asarray(inputs["captions"])
    emb = np.asarray(inputs["embed_table"])
    fpad = np.zeros((LP, B, F), np.float32)
    fpad[:L] = feats.transpose(2, 0, 1)
    # 8-wide score layout: partition p8 = (l%16)*8 + b%8, col blocks (m=l//16,
    # g=b//8); featsT8 (512, [kt is rows][m][g][p8])
    featsT = np.ascontiguousarray(
        fpad.reshape(4, 16, 4, 8, 512).transpose(4, 0, 2, 1, 3)
        .reshape(512, 2048))
    h0 = np.tanh(feats.mean(axis=2) @ f32(inputs["W_init"]).T + f32(inputs["b_init"]))
    h0T = h0.T                                         # (1024, 32)
    h0slot = np.ascontiguousarray(
        h0T.reshape(8, 128, B).transpose(1, 0, 2)).reshape(128, 256)
    h0q1, h0q2 = _qpair(h0slot.astype(NBF).astype(np.float32) * XS)
    e = f32(emb[cap])
    iw = np.concatenate([np.zeros((B, 1, F), np.float32), e[:, :-1]], axis=1)
    iwT = np.ascontiguousarray(iw.transpose(2, 1, 0)).reshape(F, T * B)
    iwq1, iwq2 = _qpair(iwT.astype(NBF).astype(np.float32) * XS)
    Wih = f32(inputs["W_ih"])
    # permute gate blocks: storage pos p holds original gate GPERM[p]
    def gperm_cols(W):                                 # W (K, 4096) -> permuted
        return np.ascontiguousarray(
            W.reshape(-1, 4, 1024)[:, GPERM].reshape(-1, 4096))
    Whh = gperm_cols(f32(inputs["W_hh"]).T)            # (1024, 4096)
    Wi1 = gperm_cols(Wih[:, :F].T)                     # (512, 4096)
    Wi2 = gperm_cols(Wih[:, F:].T)                     # (512, 4096)
    bg = (f32(inputs["b_ih"]) + f32(inputs["b_hh"])).reshape(4, 8, 128)[GPERM]
    indic32 = np.zeros((32, 1024), np.float32)
    for k in range(32):
        indic32[k, k * 32:(k + 1) * 32] = 1.0
    # maskE8 cols (g, m, c8): diag selector c == p8%8
    maskE = np.zeros((128, 128), np.float32)
    for p in range(128):
        for g in range(4):
            for m in range(4):
                maskE[p, g * 32 + m * 8 + (p % 8)] = 1.0
    # padT8 cols (g, m): pad where l = 16m + p8//8 >= L
    padT = np.zeros((128, 16), np.float32)
    for p in range(128):
        for g in range(4):
            for m in range(4):
                if 16 * m + p // 8 >= L:
                    padT[p, g * 4 + m] = -30000.0
    Pg = np.zeros((128, 8), np.float32)
    for p in range(128):
        Pg[p, p % 8] = 1.0
    bdec = (f32(inputs["b_h2o"]) + f32(inputs["b_c2o"])).reshape(4, 128).T
    shared = {
        "featsT": featsT.astype(NBF),
        "h0slot": h0slot.astype(NBF), "c0": h0slot.astype(np.float32),
        "h0q1": h0q1, "h0q2": h0q2,
        "iwq1": iwq1, "iwq2": iwq2,
        "ones32": np.ones((8, 128), np.float32),
        "Wa": f32(inputs["Wa"]).astype(NBF),           # (1024, 512) lhsT
        # gate weights pair-quantized e4m3 at xWSCALE (DoubleRow matmuls);
        # rhs x-side pair-quantized at xXS; activation scale 1/(WSCALE*XS)
        "biasLhs": (bg.reshape(32, 128) * WSCALE * XS).astype(NBF),
        "indic32": indic32.astype(NBF),
        "Wh2o": f32(inputs["W_h2o"]).T.astype(NBF),    # (1024, 512)
        # ctxh carries ctx*128 (for the e3m4 Wi1 path); Wc2o compensates
        "Wc2o": (f32(inputs["W_c2o"]).T / 128.0).astype(NBF),  # (512, 512)
        "bdec": np.ascontiguousarray(bdec),
        "maskE": maskE.astype(NBF), "padT": padT,
        "Pg": Pg,
        "ident": np.eye(128, dtype=np.float32).astype(NBF),
        "identS": (np.eye(128, dtype=np.float32) / XS).astype(NBF),
    }
    for nm, W in (("Whh", Whh), ("Wi1", Wi1), ("Wi2", Wi2)):
        if PLAN[nm] == 0:
            shared[nm] = (W * 128.0).astype(NF8)
            continue
        q1, q2 = _qpair(W * WSCALE)
        shared[nm] = q1
        if PLAN[nm] == 3:
            shared[nm + "R"] = q2
    WoutT = f32(inputs["W_out"]).T                     # (512, 32000)
    in_maps = []
    for k in range(NC):
        m = dict(shared)
        # partition-major chunk layout: row p holds [ci][kt][n] so each
        # chunk DMA is one contiguous (128, 1000-elem) slice (>=512B runs)
        m["Wout"] = np.ascontiguousarray(
            WoutT[:, VS * k:VS * (k + 1)].reshape(4, 128, 16, CW)
            .transpose(1, 2, 0, 3).reshape(128, 64 * CW)).astype(NBF)
        in_maps.append(m)
    return in_maps


def _load_tiled(nc, pool, dram, KT, N, dtype, name):
    """dram (KT*128, N) -> sbuf (128, KT*N), col block kt holds rows kt*128.."""
    t = pool.tile([128, KT * N], dtype, name=name)
    src = dram[:].rearrange("(r p) n -> p r n", p=128)
    dst = t[:].rearrange("p (r n) -> p r n", n=N)
    nc.sync.dma_start(dst, src)
    return t


def build(repeat=1, dbg=False):
    nc = bacc.Bacc("TRN2", target_bir_lowering=False, debug=False, num_devices=NC)
    di = lambda nm, sh, dt: nc.dram_tensor(nm, list(sh), dt, kind="ExternalInput")
    featsT_d = di("featsT", (512, 2048), BF)
    ones32_d = di("ones32", (8, 128), F32)
    h0_d = di("h0slot", (128, 256), BF)
    c0_d = di("c0", (128, 256), F32)
    F8E4 = mybir.dt.float8e4
    h0q1_d = di("h0q1", (128, 256), F8E4)
    h0q2_d = di("h0q2", (128, 256), F8E4)
    iwq1_d = di("iwq1", (512, 640), F8E4)
    iwq2_d = di("iwq2", (512, 640), F8E4)
    Wa_d = di("Wa", (1024, 512), BF)
    F8 = mybir.dt.float8e3
    Whh_d = di("Whh", (1024, 4096), F8E4)
    Wi1_d = di("Wi1", (512, 4096), F8)
    Wi2_d = di("Wi2", (512, 4096), F8E4)
    WhhR_d = di("WhhR", (1024, 4096), F8E4)
    biasLhs_d = di("biasLhs", (32, 128), BF)
    indic32_d = di("indic32", (32, 1024), BF)
    Wh2o_d = di("Wh2o", (1024, 512), BF)
    Wc2o_d = di("Wc2o", (512, 512), BF)
    bdec_d = di("bdec", (128, 4), F32)
    Wout_d = di("Wout", (128, 64 * CW), BF)
    maskE_d = di("maskE", (128, 128), BF)
    padT_d = di("padT", (128, 16), F32)
    Pg_d = di("Pg", (128, 8), F32)

    ident_d = di("ident", (128, 128), BF)
    identS_d = di("identS", (128, 128), BF)
    out_d = nc.dram_tensor("out", [T * B, VS], BF, kind="ExternalOutput")
    if dbg:
        dbg_hist = nc.dram_tensor("dbg_hist", [128, 8 * 256], BF, kind="ExternalOutput")
        dbg_ctxh = nc.dram_tensor("dbg_ctxh", [128, 8 * 128], BF, kind="ExternalOutput")
        dbg_cT = nc.dram_tensor("dbg_cT", [128, 256], F32, kind="ExternalOutput")

    AF = mybir.ActivationFunctionType
    AX = mybir.AxisListType
    ALU = mybir.AluOpType
    with tile.TileContext(nc) as tc:
        with tc.tile_pool(name="cst", bufs=1) as cst, \
             tc.tile_pool(name="wk", bufs=3) as wk, \
             tc.tile_pool(name="wkd", bufs=3) as wkd, \
             tc.tile_pool(name="wkv", bufs=2) as wkv, \
             tc.tile_pool(name="psg", bufs=2, space="PSUM") as psg, \
             tc.tile_pool(name="psv", bufs=2, space="PSUM") as psv, \
             tc.tile_pool(name="psps", bufs=1, space="PSUM") as psps, \
             tc.tile_pool(name="psm", bufs=1, space="PSUM") as psm:
            # ---- persistent SBUF ----
            hist = cst.tile([128, 8 * 256], BF, name="hist")
            cT = cst.tile([128, 256], F32, name="cT")
            # Wa first (u needs it + h0; the half-split lets the r-phased u
            # accumulation start on half 1), then the tiny h0/c0/hq seeds
            Wa = cst.tile([128, 8 * 512], BF, name="Wa")
            for h_ in range(2):
                nc.sync.dma_start(
                    Wa[:, h_ * 4 * 512:(h_ + 1) * 4 * 512]
                    .rearrange("p (r n) -> p r n", n=512),
                    Wa_d[h_ * 512:(h_ + 1) * 512, :]
                    .rearrange("(r p) n -> p r n", p=128))
            nc.sync.dma_start(hist[:, 7 * 256:8 * 256], h0_d[:])
            nc.sync.dma_start(cT[:], c0_d[:])
            # per-step e4m3 pair quantizations of h (persistent, rewritten
            # each step; seeded with h0 pairs -- must land before step-0 gates)
            hq1 = cst.tile([128, 256], F8E4, name="hq1")
            hq2 = cst.tile([128, 256], F8E4, name="hq2")
            nc.sync.dma_start(hq1[:], h0q1_d[:])
            nc.sync.dma_start(hq2[:], h0q2_d[:])
            featsT = cst.tile([128, 4 * 2048], BF, name="featsT")
            for kt_ in range(4):
                nc.sync.dma_start(
                    featsT[:, kt_ * 2048:(kt_ + 1) * 2048],
                    featsT_d[kt_ * 128:(kt_ + 1) * 128, :])
            small = [("biasLhs", biasLhs_d, [32, 128], BF),
                     ("indic32", indic32_d, [32, 1024], BF),
                     ("bdec", bdec_d, [128, 4], F32),
                     ("maskE", maskE_d, [128, 128], BF),
                     ("padT", padT_d, [128, 16], F32),
                     ("Pg", Pg_d, [128, 8], F32),
                     ("ones32", ones32_d, [8, 128], F32),
                     ("ident", ident_d, [128, 128], BF),
                     ("identS", identS_d, [128, 128], BF)]
            sm = {}
            for nm, d, sh, dt in small:
                sm[nm] = cst.tile(sh, dt, name=nm)
                nc.sync.dma_start(sm[nm][:], d[:])
            # Whh split into quarters: each k-pair's gates start as its
            # 1MB chunk lands at startup
            Whh = cst.tile([128, 8 * 4096], F8E4, name="Whh")
            for h_ in range(4):
                nc.sync.dma_start(
                    Whh[:, h_ * 2 * 4096:(h_ + 1) * 2 * 4096]
                    .rearrange("p (r n) -> p r n", n=4096),
                    Whh_d[h_ * 256:(h_ + 1) * 256, :]
                    .rearrange("(r p) n -> p r n", p=128))
            # fblk derived on-chip from featsT (PE transposes + DVE copies)
            # instead of a second 2MB DMA of the same data
            fblk = cst.tile([128, 16 * 512], BF, name="fblk")
            # Wi1 split into kt-halves: step-0 wi1 phase 0 starts ~3us sooner
            Wi1 = cst.tile([128, 4 * 4096], F8, name="Wi1")
            for h_ in range(2):
                nc.sync.dma_start(
                    Wi1[:, h_ * 2 * 4096:(h_ + 1) * 2 * 4096]
                    .rearrange("p (r n) -> p r n", n=4096),
                    Wi1_d[h_ * 256:(h_ + 1) * 256, :]
                    .rearrange("(r p) n -> p r n", p=128))
            # Wi2/iwq needed first at t=1 (iw_0 is zeros): load after Wi1
            iwq1 = _load_tiled(nc, cst, iwq1_d, 4, 640, F8E4, "iwq1")
            iwq2 = _load_tiled(nc, cst, iwq2_d, 4, 640, F8E4, "iwq2")
            # Wi2 in halves: t=1's gates_iw pair 0 starts on half 1
            Wi2 = cst.tile([128, 4 * 4096], F8E4, name="Wi2")
            for h_ in range(2):
                nc.sync.dma_start(
                    Wi2[:, h_ * 2 * 4096:(h_ + 1) * 2 * 4096]
                    .rearrange("p (r n) -> p r n", n=4096),
                    Wi2_d[h_ * 256:(h_ + 1) * 256, :]
                    .rearrange("(r p) n -> p r n", p=128))
            Wh2o = _load_tiled(nc, cst, Wh2o_d, 8, 512, BF, "Wh2o")
            Wc2o = _load_tiled(nc, cst, Wc2o_d, 4, 512, BF, "Wc2o")
            WoutS = cst.tile([128, 64 * CW], BF, name="WoutS")
            nc.sync.dma_start(WoutS[:], Wout_d[:])
            # Whh residual (Q2) loads last; steps t < RSKIP run Q1-only
            WhhR = _load_tiled(nc, cst, WhhR_d, 8, 4096, F8E4, "WhhR")
            biasLhs, indic32, bdec = sm["biasLhs"], sm["indic32"], sm["bdec"]
            maskE, padT, Pg, ones32, ident, identS = (
                sm["maskE"], sm["padT"], sm["Pg"], sm["ones32"], sm["ident"],
                sm["identS"])
            ctxh = cst.tile([128, 8 * 128], BF, name="ctxh")
            # decT ring of 2 groups: col kt*256 + (g%2)*128 + (t%4)*32 + b
            decT = cst.tile([128, 4 * 256], BF, name="decT")
            mm = nc.tensor.matmul

            def gblk(gp, bi):
                # gates stored in two bank-sized tiles (separate dependency
                # tracking): A = {g,f} blocks bi 0-15, B = {i,o} blocks 16-31
                if bi < 16:
                    return gp[0][:, bi * 32:(bi + 1) * 32]
                return gp[1][:, (bi - 16) * 32:(bi - 15) * 32]

            DRM = mybir.MatmulPerfMode.DoubleRow
            Wi2v = Wi2[:].rearrange("p (pr two n) -> p pr two n", pr=2, two=2)
            Whhv = Whh[:].rearrange("p (pr two n) -> p pr two n", pr=4, two=2)
            WhhRv = WhhR[:].rearrange("p (pr two n) -> p pr two n", pr=4, two=2)
            iwq1v = iwq1[:].rearrange("p (kt n) -> p kt n", kt=4)
            iwq2v = iwq2[:].rearrange("p (kt n) -> p kt n", kt=4)
            hq1v = hq1[:].rearrange("p (pr two n) -> p pr two n", pr=4, two=2)
            hq2v = hq2[:].rearrange("p (pr two n) -> p pr two n", pr=4, two=2)

            def gates_iw(t):
                _sec(nc, 'gatesiw')
                """allocate gps pair for step t; bias + iw part (no h/ctx dep)"""
                gp = (psg.tile([128, 512], F32, name="gpsA", tag="ga"),
                      psg.tile([128, 512], F32, name="gpsB", tag="gb"))
                mm(gp[0][:], biasLhs[:], indic32[:, 0:512],
                   start=True, stop=False)
                mm(gp[1][:], biasLhs[:], indic32[:, 512:1024],
                   start=True, stop=False)
                if t == 0:
                    # iw_0 is exact zeros by construction: its matmuls are an
                    # identity no-op; skipping them lets Wi2/iwq load after
                    # Wi1 (off the startup critical path)
                    return gp
                for pr in range(2):
                    for bi in range(32):
                        blk = gblk(gp, bi)
                        w1 = Wi2v[:, pr, :, bi * 128:(bi + 1) * 128]
                        for xv in (iwq1v, iwq2v):
                            mm(blk, w1,
                               xv[:, 2 * pr:2 * pr + 2, t * 32:(t + 1) * 32],
                               start=False, stop=False, perf_mode=DRM)
                return gp

            def dec_group(gq):
                _sec(nc, 'dec')
                # Wh2o/Wc2o lhsT tiles streamed from HBM per group
                s0 = (4 * gq) % 8
                g2 = gq % 2
                hv = hist[:].rearrange("p (s r b) -> p s r b", s=8, b=32)
                cv = ctxh[:].rearrange("p (s r b) -> p s r b", s=8, b=32)
                dps = psm.tile([128, 512], F32, name="dps", tag="x")
                for r in range(8):
                    for m in range(4):
                        mm(dps[:, m * 128:(m + 1) * 128],
                           Wh2o[:, r * 512 + m * 128: r * 512 + (m + 1) * 128],
                           hv[:, s0:s0 + 4, r, :],
                           start=(r == 0 and m == 0), stop=False)
                for r in range(4):
                    for m in range(4):
                        mm(dps[:, m * 128:(m + 1) * 128],
                           Wc2o[:, r * 512 + m * 128: r * 512 + (m + 1) * 128],
                           cv[:, s0:s0 + 4, r, :], start=False, stop=False)
                for m in range(4):
                    mm(dps[:, m * 128:(m + 1) * 128], identS[:],
                       iwq1[:, m * 640 + gq * 128: m * 640 + (gq + 1) * 128],
                       start=False, stop=False)
                for m in range(4):
                    mm(dps[:, m * 128:(m + 1) * 128], identS[:],
                       iwq2[:, m * 640 + gq * 128: m * 640 + (gq + 1) * 128],
                       start=False, stop=True)
                for m in range(4):
                    nc.scalar.activation(
                        decT[:, m * 256 + g2 * 128: m * 256 + (g2 + 1) * 128],
                        dps[:, m * 128:(m + 1) * 128], AF.Tanh,
                        bias=bdec[:, m:m + 1])

            def vocab_chunks(gq, cis, fine=False):
                # consecutive chunks paired into one output DMA so the store
                # has >=512B contiguous runs (sub-512B runs pay 2x latency);
                # fine=True (endgame drain) goes chunk-at-a-time so the
                # mm->copy->store pipeline is twice as deep
                _sec(nc, 'vocab')
                g2 = gq % 2
                cis = list(cis)
                i = 0
                while i < len(cis):
                    pair = cis[i:i + 1] if fine else cis[i:i + 2]
                    if len(pair) == 2 and pair[1] != pair[0] + 1:
                        pair = pair[:1]
                    lgv = wkv.tile([128, 2 * CW], BF, name="lgv")
                    vps = psv.tile([128, 2 * CW], F32, name="vps")
                    for j, ci in enumerate(pair):
                        for kt in range(4):
                            mm(vps[:, j * CW:(j + 1) * CW],
                               decT[:, kt * 256 + g2 * 128: kt * 256 + (g2 + 1) * 128],
                               WoutS[:, ci * 4 * CW + kt * CW: ci * 4 * CW + (kt + 1) * CW],
                               start=(kt == 0), stop=(kt == 3))
                    # GPSIMD can't read PSUM on HW; alternate ACT/DVE to
                    # spread the eviction load across both engines
                    if (pair[0] // 2) % 2 == 0:
                        nc.scalar.copy(lgv[:, 0:len(pair) * CW],
                                       vps[:, 0:len(pair) * CW])
                    else:
                        nc.vector.tensor_copy(lgv[:, 0:len(pair) * CW],
                                              vps[:, 0:len(pair) * CW])
                    nc.sync.dma_start(
                        out_d[gq * 128:(gq + 1) * 128,
                              pair[0] * CW: pair[0] * CW + len(pair) * CW],
                        lgv[:, 0:len(pair) * CW])
                    i += len(pair)

            for rep in range(repeat):
                if rep > 0:
                    nc.sync.dma_start(hist[:, 7 * 256:8 * 256], h0_d[:])
                    nc.sync.dma_start(cT[:], c0_d[:])
                gps = None
                for t in range(T):
                    s = (t - 1) % 8
                    w = t % 8
                    hs = lambda kt: hist[:, s * 256 + kt * 32: s * 256 + kt * 32 + 32]
                    # -- u = Wa^T h  (512, 32) as (128, 4x32)
                    _sec(nc, 'u')
                    pu = psm.tile([128, 128], F32, name="pu", tag="x")
                    for rh in range(2):
                        for m in range(4):
                            for r in range(rh * 4, rh * 4 + 4):
                                mm(pu[:, m * 32:(m + 1) * 32],
                                   Wa[:, r * 512 + m * 128: r * 512 + (m + 1) * 128],
                                   hs(r), start=(rh == 0 and m == 0 and r == 0),
                                   stop=(r == 7))
                    u = wk.tile([128, 128], BF, name="u", bufs=2)
                    nc.scalar.copy(u[:], pu[:])
                    # -- scoresT (2048, 32) as (128, 16x32)
                    _sec(nc, 'scoresT')
                    sps = psps.tile([128, 128], F32, name="sps")
                    for kt in range(4):
                        for g in range(4):
                            for m in range(4):
                                mm(sps[:, g * 32 + m * 8: g * 32 + m * 8 + 8],
                                   featsT[:, kt * 2048 + (m * 4 + g) * 128:
                                          kt * 2048 + (m * 4 + g) * 128 + 128],
                                   u[:, kt * 32 + g * 8: kt * 32 + g * 8 + 8],
                                   start=(kt == 0 and g == 0 and m == 0),
                                   stop=(kt == 3))
                    if t == 0:
                        # derive fblk = featsT^T block-wise (PE transpose via
                        # identity rhs, Pool copies PSUM->SBUF); runs in the
                        # startup DMA window, saves a 2MB HBM load
                        _sec(nc, 'transp')
                        for mg in range(16):
                            tps = psv.tile([128, 4 * 128], BF, name="tps",
                                           tag="vps")
                            for kt in range(4):
                                mm(tps[:, kt * 128:(kt + 1) * 128],
                                   featsT[:, kt * 2048 + mg * 128:
                                          kt * 2048 + mg * 128 + 128],
                                   ident[:], is_transpose=True,
                                   start=True, stop=True)
                            nc.vector.tensor_copy(
                                fblk[:, mg * 512:(mg + 1) * 512], tps[:])
                    # -- gates bias+iw part (t=0 only; later steps emit it
                    # at the end of the previous step to fill the LSTM gap)
                    _sec(nc, 'gates0')
                    if gps is None:
                        gps = gates_iw(0)
                    _sec(nc, 'gatesWhh')
                    # -- gates h part: DR 3-term (Q1w*x1, Q2w*x1, Q1w*x2),
                    # pair-phased so the first Whh DMA half serves step 0.
                    # Steps 0-1 skip the Q2w residual so its 4MB DMA can land
                    # off the startup critical path (err contribution tested
                    # in acc_model: negligible).
                    for prh in range(4):
                        for bi in range(32):
                            blk = gblk(gps, bi)
                            for pr in range(prh, prh + 1):
                                x1 = hq1v[:, pr, :, :]
                                mm(blk, Whhv[:, pr, :, bi * 128:(bi + 1) * 128],
                                   x1, start=False, stop=False, perf_mode=DRM)
                                if t >= RSKIP:
                                    mm(blk, WhhRv[:, pr, :, bi * 128:(bi + 1) * 128],
                                       x1, start=False, stop=False, perf_mode=DRM)
                                mm(blk, Whhv[:, pr, :, bi * 128:(bi + 1) * 128],
                                   hq2v[:, pr, :, :],
                                   start=False, stop=False, perf_mode=DRM)
                    _sec(nc, 'dec+v2')
                    # deferred dec for the previous group
                    if t % 4 == 0 and t > 0:
                        dec_group(t // 4 - 1)
                    # 2 vocab chunks fill the softmax window
                    if t >= 4:
                        _lo, _hi = [(0, 1), (1, 6), (6, 11), (11, 16)][t % 4]
                        _mid = _lo + (1 if t % 4 == 0 else 2)
                        vocab_chunks(t // 4 - 1, range(_lo, _mid))
                    # -- diag extract + softmax (no max-sub; |scores| < 88)
                    _sec(nc, 'softmax')
                    nc.vector.tensor_mul(sps[:], sps[:], maskE[:])
                    sd = wk.tile([128, 16], F32, name="sd", bufs=2)
                    nc.vector.reduce_sum(
                        sd[:], sps[:].rearrange("p (gm c) -> p gm c", c=8), axis=AX.X)
                    nc.vector.tensor_add(sd[:], sd[:], padT[:])
                    ex = wk.tile([128, 16], BF, name="ex", bufs=2)
                    nc.scalar.activation(ex[:], sd[:], AF.Exp)
                    rows = wk.tile([128, 4], F32, name="rows", bufs=2)
                    nc.vector.reduce_sum(
                        rows[:], ex[:].rearrange("p (g m) -> p g m", m=4),
                        axis=AX.X)
                    # dummy: pulls the sigmoid-set table load (1.3us) into the
                    # post-exp window instead of the LSTM critical chain
                    dum = wk.tile([128, 1], F32, name="dum")
                    nc.scalar.activation(dum[:], rows[:, 0:1], AF.Sigmoid)
                    if HEAT_A:
                        heat = psm.tile([128, 32], F32, name="heat", tag="x")
                        for _hk in range(HEAT_A):
                            mm(heat[:], Wa[:, 0:128], Wa[:, 0:32],
                               start=True, stop=True)
                    # unnormalized diag expansion FIRST: aEs = ex * maskE is
                    # the only gate for ctx; the 1/sum normalizer (pS/rS/rbB)
                    # is built while the ctx matmuls run and lands in rbBs for
                    # the eviction stt. pS/rbB live in the sps PSUM ring
                    # (dead after the mask-mul) to avoid a tag-x deadlock.
                    aEs = wk.tile([128, 128], BF, name="aEs", bufs=2)
                    nc.vector.scalar_tensor_tensor(
                        aEs[:].rearrange("p (gm c) -> p gm c", c=8),
                        ex[:].rearrange("p gm -> p gm ()").broadcast_to([128, 16, 8]),
                        1.0,
                        maskE[:].rearrange("p (gm c) -> p gm c", c=8),
                        ALU.mult, ALU.mult)
                    # -- ctxT (512, 32) as (128, 4x32)
                    _sec(nc, 'ctx')
                    cps = psm.tile([128, 128], F32, name="cps", tag="x")
                    for mf in range(4):
                        for g in range(4):
                            for ml in range(4):
                                mm(cps[:, mf * 32 + g * 8: mf * 32 + g * 8 + 8],
                                   fblk[:, (ml * 4 + g) * 512 + mf * 128:
                                        (ml * 4 + g) * 512 + mf * 128 + 128],
                                   aEs[:, g * 32 + ml * 8: g * 32 + ml * 8 + 8],
                                   start=(ml == 0), stop=(ml == 3))
                    # normalizer, hidden behind the ctx matmuls
                    _sec(nc, 'softmax')
                    pS = psps.tile([8, 4], F32, name="pS", tag="sps")
                    mm(pS[:], Pg[:], rows[:], start=True, stop=True)
                    rS = wk.tile([8, 4], F32, name="rS")
                    nc.vector.reciprocal(rS[:], pS[:])
                    # rSd cols (g, c): rS[p8, g] * (p8 == c), b-ordered
                    rSd = wk.tile([8, 32], F32, name="rSd")
                    nc.vector.scalar_tensor_tensor(
                        rSd[:].rearrange("p (g c) -> p g c", c=8),
                        rS[:].rearrange("p g -> p g ()").broadcast_to([8, 4, 8]),
                        128.0,
                        Pg[0:8, 0:8].rearrange("p c -> p () c").broadcast_to([8, 4, 8]),
                        ALU.mult, ALU.mult)
                    rbB = psps.tile([128, 32], F32, name="rbB", tag="sps")
                    mm(rbB[:], ones32[:], rSd[:], start=True, stop=True)
                    # HW: DVE may read only ONE non-scalar PSUM input; the
                    # norm stt also reads cps, so rbB must bounce via SBUF
                    rbBs = wk.tile([128, 32], F32, name="rbBs")
                    nc.vector.tensor_copy(rbBs[:], rbB[:])
                    _sec(nc, 'ctx')
                    # normalize while evicting: ctxh = cps * rbBs
                    nc.vector.scalar_tensor_tensor(
                        ctxh[:, w * 128:(w + 1) * 128]
                        .rearrange("p (k c) -> p k c", c=32),
                        cps[:].rearrange("p (k c) -> p k c", c=32), 1.0,
                        rbBs[:].rearrange("p c -> p () c").broadcast_to([128, 4, 32]),
                        ALU.mult, ALU.mult)
                    # -- gates ctx part: bank0 gates (g,f) first, their
                    # activations + c-mul overlap the bank1 (i,o) matmuls
                    # (bank-aligned so no PSUM-bank ping-pong)
                    _sec(nc, 'Wi1+act')
                    gG, gF, gI, gO = (gps[0][:, 0:256], gps[0][:, 256:512],
                                      gps[1][:, 0:256], gps[1][:, 256:512])
                    tGs = wk.tile([128, 256], BF, name="tGs", bufs=2)
                    def wi1_sec(gsec):
                        for bi in range(gsec * 8, gsec * 8 + 8):
                            blk = gblk(gps, bi)
                            for kt in range(4):
                                mm(blk, Wi1[:, kt * 4096 + bi * 128: kt * 4096 + (bi + 1) * 128],
                                   ctxh[:, w * 128 + kt * 32: w * 128 + (kt + 1) * 32],
                                   start=False, stop=(kt == 3))
                    wi1_sec(0)
                    wi1_sec(1)
                    wi1_sec(2)
                    wi1_sec(3)
                    gps_next = gates_iw(t + 1) if t + 1 < T else None
                    _sec(nc, 'Wi1+act')
                    nc.scalar.activation(tGs[:], gG, AF.Tanh, scale=1.0 / (WSCALE * XS))
                    sF = wk.tile([128, 256], BF, name="sF", bufs=2)
                    nc.scalar.activation(sF[:], gF, AF.Sigmoid, scale=1.0 / (WSCALE * XS))
                    nc.vector.tensor_mul(cT[:], cT[:], sF[:])
                    # gate activations land in SBUF bf16: frees the gps PSUM
                    # bank early and enables 2x DVE on the i*g / o*tanh muls
                    gIs = wk.tile([128, 256], BF, name="gIs", bufs=2)
                    nc.scalar.activation(gIs[:], gI, AF.Sigmoid, scale=1.0 / (WSCALE * XS))
                    gOs = wk.tile([128, 256], BF, name="gOs", bufs=2)
                    nc.scalar.activation(gOs[:], gO, AF.Sigmoid, scale=1.0 / (WSCALE * XS))
                    # -- LSTM-gap fillers: next step's dep-free gate matmuls
                    # first (no DMA dependency), then 2 more vocab chunks
                    _sec(nc, 'giw+v2b')
                    # -- p-state heater: dep-free dummy matmuls keep the PE
                    # clock streak alive through the LSTM tail (a broken
                    # streak costs ~3us of half-speed ramp-up)
                    if HEAT_B and t >= 15:
                        heat2 = psm.tile([128, 32], F32, name="heat", tag="x")
                        for _hk in range(HEAT_B):
                            mm(heat2[:], Wa[:, 0:128], Wa[:, 0:32],
                               start=True, stop=True)
                    # -- LSTM elementwise tail (activations emitted above)
                    _sec(nc, 'lstmtail')
                    nc.vector.tensor_mul(gIs[:], gIs[:], tGs[:])
                    nc.vector.tensor_add(cT[:], cT[:], gIs[:])
                    tCs = wk.tile([128, 256], BF, name="tCs", bufs=2)
                    nc.scalar.activation(tCs[:, 0:128], cT[:, 0:128], AF.Tanh)
                    nc.vector.tensor_mul(hist[:, w * 256: w * 256 + 128],
                                         gOs[:, 0:128], tCs[:, 0:128])
                    nc.scalar.activation(tCs[:, 128:256], cT[:, 128:256], AF.Tanh)
                    nc.vector.tensor_mul(hist[:, w * 256 + 128:(w + 1) * 256],
                                         gOs[:, 128:256], tCs[:, 128:256])
                    # e4m3 pair quantization of h for next step's DR gates
                    hsl = hist[:, w * 256:(w + 1) * 256]
                    nc.scalar.activation(hq1[:], hsl, AF.Copy, scale=XS)
                    nc.vector.scalar_tensor_tensor(
                        hq2[:], hsl, XS, hq1[:], ALU.mult, ALU.subtract)
                    # tail-slot vocab emitted AFTER the tail ops: their PE
                    # matmuls still fill this window, but the PSUM->SBUF
                    # copies now queue BEHIND the critical tail ACT/DVE ops
                    # instead of blocking them (in-order engine queues)
                    if t >= 4:
                        vocab_chunks(t // 4 - 1, range(_mid, _hi))
                    gps = gps_next
                # tail: last dec group + its vocab
                dec_group(4)
                vocab_chunks(4, range(16))
                if dbg:
                    nc.sync.dma_start(dbg_hist[:], hist[:])
                    nc.sync.dma_start(dbg_ctxh[:], ctxh[:])
                    nc.sync.dma_start(dbg_cT[:], cT[:])
    nc.finalize()
    return nc


def kernel(**inputs) -> np.ndarray:
    if "nc" not in _BUILT:
        _BUILT["nc"] = build()
    nc = _BUILT["nc"]
    in_maps = host_prep(inputs)
    res = run_bass_kernel_spmd(nc, in_maps, core_ids=list(range(NC)))
    full = np.concatenate(
        [np.asarray(res.results[k]["out"]) for k in range(NC)], axis=1)
    # (640, 32000) bf16, row t*32+b -> (B, T, V) f32
    out = np.ascontiguousarray(
        full.reshape(T, B, V).transpose(1, 0, 2)).astype(np.float32)
    b_out = np.asarray(inputs["b_out"], np.float32)
    if np.any(b_out):
        out += b_out[None, None, :]
    return out



# revision 93
# speedup vs baseline: 1.0148x; 1.0001x over previous
"""Trainium2 Bass kernel for nn_AttnDecoderRNN (B=32,T=20,L=49,F=512,H=1024,V=32000).

Zero-collective design across 8 NeuronCores:
- The attention-LSTM recurrence is fully REPLICATED on every core (tensor-
  parallel splits need a per-step AllGather of h, which dominates cost).
- Only the vocab projection is tensor-parallel: core k owns W_out rows
  [4000k, 4000(k+1)) and emits a (640, 4000) bf16 logit shard; the host
  concatenation is the all-gather. b_out is added host-side (skipped if 0).

Per-core schedule (all layouts feature-on-partition):
- scoresT attention: sps[(l,b), b'] = feats . (Wa^T h), diagonal extracted
  with a one-hot mask + grouped reduce; softmax runs in the (l%4 x b, l//4)
  layout with tiny PE sum/broadcast matmuls; normalize+expand fused into one
  scalar_tensor_tensor.
- gates: one PSUM accumulation (bias matmul + iw + h + ctx parts); the
  LSTM elementwise runs mostly in place on PSUM; a dummy sigmoid after the
  softmax Exp prefetches the ACT table set off the critical chain.
- dec groups (4 steps) with Wh2o/Wc2o streamed from HBM; vocab chunks are
  interleaved into the step loop (W_out streamed) to keep the PE fed
  through the softmax/LSTM dependency-chain windows.
"""
import sys

sys.path.insert(0, "/opt/trn_rl_repo")
import numpy as np
import ml_dtypes

import concourse.bass as bass
import concourse.mybir as mybir
import concourse.tile as tile
from concourse import bacc
from concourse.bass_utils import run_bass_kernel_spmd

B, T, L, F, H, V = 32, 20, 49, 512, 1024, 32000
LP = 64
MB = 13            # l-blocks kept (l < 4*MB covers L=49; blocks 13-15 are all-pad)
NC = 8
VS = V // NC      # 4000
CW = 250          # vocab chunk width (16 chunks per group)
BF = mybir.dt.bfloat16
F32 = mybir.dt.float32
NBF = ml_dtypes.bfloat16
NF8 = ml_dtypes.float8_e3m4
NE4 = ml_dtypes.float8_e4m3
WSCALE = 512.0         # gate weight pre-scale (e4m3 normal range)
XS = 32.0              # gate rhs (h/ctx/iw) pre-scale for e4m3
GPERM = [2, 1, 0, 3]   # gate storage g,f,i,o: bank0={g,f} early, bank1={i,o} late
HEAT_A = 0            # heater matmuls filling the softmax window
HEAT_B = 0            # heater matmuls filling the LSTM-tail window
RSKIP = 6              # steps running without the Whh Q2 residual (DMA window)
# DoubleRow fp8 gates: weights pair-quantized (Q1+Q2) where listed 3,
# single-quantized (Q1 only) where 2; rhs always pair-quantized. Each
# DR matmul covers 2 k-tiles at 0.5 cycles/row. Wi1 (value 0) stays
# plain e3m4 at x128 with a bf16 ctx rhs scaled x128 (rbBs carries the
# 128; Wc2o is pre-divided by 128 to compensate on the dec path) --
# its pair-residual would not fit SBUF.
PLAN = {"Whh": 3, "Wi1": 0, "Wi2": 2}


def _qpair(x):
    q1 = x.astype(NE4)
    q2 = (x - q1.astype(np.float32)).astype(NE4)
    return q1, q2

_BUILT = {}
SECTIONS = []


def _sec(nc, label):
    SECTIONS.append((int(nc.get_next_instruction_name().split('-')[1]), label))


def host_prep(inputs):
    f32 = lambda x: np.asarray(x, np.float32)
    feats = f32(inputs["features"])                    # (B, F, L)
    cap = np.asarray(inputs["captions"])
    emb = np.asarray(inputs["embed_table"])
    fpad = np.zeros((LP, B, F), np.float32)
    fpad[:L] = feats.transpose(2, 0, 1)
    # 8-wide score layout: partition p8 = (l%16)*8 + b%8, col blocks (m=l//16,
    # g=b//8); featsT8 (512, [kt is rows][m][g][p8])
    featsT = np.ascontiguousarray(
        fpad.reshape(4, 16, 4, 8, 512).transpose(4, 0, 2, 1, 3)
        .reshape(512, 2048))
    h0 = np.tanh(feats.mean(axis=2) @ f32(inputs["W_init"]).T + f32(inputs["b_init"]))
    h0T = h0.T                                         # (1024, 32)
    h0slot = np.ascontiguousarray(
        h0T.reshape(8, 128, B).transpose(1, 0, 2)).reshape(128, 256)
    h0q1, h0q2 = _qpair(h0slot.astype(NBF).astype(np.float32) * XS)
    e = f32(emb[cap])
    iw = np.concatenate([np.zeros((B, 1, F), np.float32), e[:, :-1]], axis=1)
    iwT = np.ascontiguousarray(iw.transpose(2, 1, 0)).reshape(F, T * B)
    iwq1, iwq2 = _qpair(iwT.astype(NBF).astype(np.float32) * XS)
    Wih = f32(inputs["W_ih"])
    # permute gate blocks: storage pos p holds original gate GPERM[p]
    def gperm_cols(W):                                 # W (K, 4096) -> permuted
        return np.ascontiguousarray(
            W.reshape(-1, 4, 1024)[:, GPERM].reshape(-1, 4096))
    Whh = gperm_cols(f32(inputs["W_hh"]).T)            # (1024, 4096)
    Wi1 = gperm_cols(Wih[:, :F].T)                     # (512, 4096)
    Wi2 = gperm_cols(Wih[:, F:].T)                     # (512, 4096)
    bg = (f32(inputs["b_ih"]) + f32(inputs["b_hh"])).reshape(4, 8, 128)[GPERM]
    indic32 = np.zeros((32, 1024), np.float32)
    for k in range(32):
        indic32[k, k * 32:(k + 1) * 32] = 1.0
    # maskE8 cols (g, m, c8): diag selector c == p8%8
    maskE = np.zeros((128, 128), np.float32)
    for p in range(128):
        for g in range(4):
            for m in range(4):
                maskE[p, g * 32 + m * 8 + (p % 8)] = 1.0
    # padT8 cols (g, m): pad where l = 16m + p8//8 >= L
    padT = np.zeros((128, 16), np.float32)
    for p in range(128):
        for g in range(4):
            for m in range(4):
                if 16 * m + p // 8 >= L:
                    padT[p, g * 4 + m] = -30000.0
    Pg = np.zeros((128, 8), np.float32)
    for p in range(128):
        Pg[p, p % 8] = 1.0
    bdec = (f32(inputs["b_h2o"]) + f32(inputs["b_c2o"])).reshape(4, 128).T
    shared = {
        "featsT": featsT.astype(NBF),
        "h0slot": h0slot.astype(NBF), "c0": h0slot.astype(np.float32),
        "h0q1": h0q1, "h0q2": h0q2,
        "iwq1": iwq1, "iwq2": iwq2,
        "ones32": np.ones((8, 128), np.float32),
        "Wa": f32(inputs["Wa"]).astype(NBF),           # (1024, 512) lhsT
        # gate weights pair-quantized e4m3 at xWSCALE (DoubleRow matmuls);
        # rhs x-side pair-quantized at xXS; activation scale 1/(WSCALE*XS)
        "biasLhs": (bg.reshape(32, 128) * WSCALE * XS).astype(NBF),
        "indic32": indic32.astype(NBF),
        "Wh2o": f32(inputs["W_h2o"]).T.astype(NBF),    # (1024, 512)
        # ctxh carries ctx*128 (for the e3m4 Wi1 path); Wc2o compensates
        "Wc2o": (f32(inputs["W_c2o"]).T / 128.0).astype(NBF),  # (512, 512)
        "bdec": np.ascontiguousarray(bdec),
        "maskE": maskE.astype(NBF), "padT": padT,
        "Pg": Pg,
        "ident": np.eye(128, dtype=np.float32).astype(NBF),
        "identS": (np.eye(128, dtype=np.float32) / XS).astype(NBF),
    }
    for nm, W in (("Whh", Whh), ("Wi1", Wi1), ("Wi2", Wi2)):
        if PLAN[nm] == 0:
            shared[nm] = (W * 128.0).astype(NF8)
            continue
        q1, q2 = _qpair(W * WSCALE)
        shared[nm] = q1
        if PLAN[nm] == 3:
            shared[nm + "R"] = q2
    WoutT = f32(inputs["W_out"]).T                     # (512, 32000)
    in_maps = []
    for k in range(NC):
        m = dict(shared)
        # partition-major chunk layout: row p holds [ci][kt][n] so each
        # chunk DMA is one contiguous (128, 1000-elem) slice (>=512B runs)
        m["Wout"] = np.ascontiguousarray(
            WoutT[:, VS * k:VS * (k + 1)].reshape(4, 128, 16, CW)
            .transpose(1, 2, 0, 3).reshape(128, 64 * CW)).astype(NBF)
        in_maps.append(m)
    return in_maps


def _load_tiled(nc, pool, dram, KT, N, dtype, name):
    """dram (KT*128, N) -> sbuf (128, KT*N), col block kt holds rows kt*128.."""
    t = pool.tile([128, KT * N], dtype, name=name)
    src = dram[:].rearrange("(r p) n -> p r n", p=128)
    dst = t[:].rearrange("p (r n) -> p r n", n=N)
    nc.sync.dma_start(dst, src)
    return t


def build(repeat=1, dbg=False):
    nc = bacc.Bacc("TRN2", target_bir_lowering=False, debug=False, num_devices=NC)
    di = lambda nm, sh, dt: nc.dram_tensor(nm, list(sh), dt, kind="ExternalInput")
    featsT_d = di("featsT", (512, 2048), BF)
    ones32_d = di("ones32", (8, 128), F32)
    h0_d = di("h0slot", (128, 256), BF)
    c0_d = di("c0", (128, 256), F32)
    F8E4 = mybir.dt.float8e4
    h0q1_d = di("h0q1", (128, 256), F8E4)
    h0q2_d = di("h0q2", (128, 256), F8E4)
    iwq1_d = di("iwq1", (512, 640), F8E4)
    iwq2_d = di("iwq2", (512, 640), F8E4)
    Wa_d = di("Wa", (1024, 512), BF)
    F8 = mybir.dt.float8e3
    Whh_d = di("Whh", (1024, 4096), F8E4)
    Wi1_d = di("Wi1", (512, 4096), F8)
    Wi2_d = di("Wi2", (512, 4096), F8E4)
    WhhR_d = di("WhhR", (1024, 4096), F8E4)
    biasLhs_d = di("biasLhs", (32, 128), BF)
    indic32_d = di("indic32", (32, 1024), BF)
    Wh2o_d = di("Wh2o", (1024, 512), BF)
    Wc2o_d = di("Wc2o", (512, 512), BF)
    bdec_d = di("bdec", (128, 4), F32)
    Wout_d = di("Wout", (128, 64 * CW), BF)
    maskE_d = di("maskE", (128, 128), BF)
    padT_d = di("padT", (128, 16), F32)
    Pg_d = di("Pg", (128, 8), F32)

    ident_d = di("ident", (128, 128), BF)
    identS_d = di("identS", (128, 128), BF)
    out_d = nc.dram_tensor("out", [T * B, VS], BF, kind="ExternalOutput")
    if dbg:
        dbg_hist = nc.dram_tensor("dbg_hist", [128, 8 * 256], BF, kind="ExternalOutput")
        dbg_ctxh = nc.dram_tensor("dbg_ctxh", [128, 8 * 128], BF, kind="ExternalOutput")
        dbg_cT = nc.dram_tensor("dbg_cT", [128, 256], F32, kind="ExternalOutput")

    AF = mybir.ActivationFunctionType
    AX = mybir.AxisListType
    ALU = mybir.AluOpType
    with tile.TileContext(nc) as tc:
        with tc.tile_pool(name="cst", bufs=1) as cst, \
             tc.tile_pool(name="wk", bufs=3) as wk, \
             tc.tile_pool(name="wkd", bufs=3) as wkd, \
             tc.tile_pool(name="wkv", bufs=2) as wkv, \
             tc.tile_pool(name="psg", bufs=2, space="PSUM") as psg, \
             tc.tile_pool(name="psv", bufs=2, space="PSUM") as psv, \
             tc.tile_pool(name="psps", bufs=1, space="PSUM") as psps, \
             tc.tile_pool(name="psm", bufs=1, space="PSUM") as psm:
            # ---- persistent SBUF ----
            hist = cst.tile([128, 8 * 256], BF, name="hist")
            cT = cst.tile([128, 256], F32, name="cT")
            # Wa first (u needs it + h0; the half-split lets the r-phased u
            # accumulation start on half 1), then the tiny h0/c0/hq seeds
            Wa = cst.tile([128, 8 * 512], BF, name="Wa")
            for h_ in range(2):
                nc.sync.dma_start(
                    Wa[:, h_ * 4 * 512:(h_ + 1) * 4 * 512]
                    .rearrange("p (r n) -> p r n", n=512),
                    Wa_d[h_ * 512:(h_ + 1) * 512, :]
                    .rearrange("(r p) n -> p r n", p=128))
            nc.sync.dma_start(hist[:, 7 * 256:8 * 256], h0_d[:])
            nc.sync.dma_start(cT[:], c0_d[:])
            # per-step e4m3 pair quantizations of h (persistent, rewritten
            # each step; seeded with h0 pairs -- must land before step-0 gates)
            hq1 = cst.tile([128, 256], F8E4, name="hq1")
            hq2 = cst.tile([128, 256], F8E4, name="hq2")
            nc.sync.dma_start(hq1[:], h0q1_d[:])
            nc.sync.dma_start(hq2[:], h0q2_d[:])
            featsT = cst.tile([128, 4 * 2048], BF, name="featsT")
            for kt_ in range(4):
                nc.sync.dma_start(
                    featsT[:, kt_ * 2048:(kt_ + 1) * 2048],
                    featsT_d[kt_ * 128:(kt_ + 1) * 128, :])
            small = [("biasLhs", biasLhs_d, [32, 128], BF),
                     ("indic32", indic32_d, [32, 1024], BF),
                     ("bdec", bdec_d, [128, 4], F32),
                     ("maskE", maskE_d, [128, 128], BF),
                     ("padT", padT_d, [128, 16], F32),
                     ("Pg", Pg_d, [128, 8], F32),
                     ("ones32", ones32_d, [8, 128], F32),
                     ("ident", ident_d, [128, 128], BF),
                     ("identS", identS_d, [128, 128], BF)]
            sm = {}
            for nm, d, sh, dt in small:
                sm[nm] = cst.tile(sh, dt, name=nm)
                nc.sync.dma_start(sm[nm][:], d[:])
            # Whh split into halves: step-0 gates on kt 0-3 start ~6us sooner
            Whh = cst.tile([128, 8 * 4096], F8E4, name="Whh")
            for h_ in range(2):
                nc.sync.dma_start(
                    Whh[:, h_ * 4 * 4096:(h_ + 1) * 4 * 4096]
                    .rearrange("p (r n) -> p r n", n=4096),
                    Whh_d[h_ * 512:(h_ + 1) * 512, :]
                    .rearrange("(r p) n -> p r n", p=128))
            # fblk derived on-chip from featsT (PE transposes + DVE copies)
            # instead of a second 2MB DMA of the same data
            fblk = cst.tile([128, 16 * 512], BF, name="fblk")
            # Wi1 split into kt-halves: step-0 wi1 phase 0 starts ~3us sooner
            Wi1 = cst.tile([128, 4 * 4096], F8, name="Wi1")
            for h_ in range(2):
                nc.sync.dma_start(
                    Wi1[:, h_ * 2 * 4096:(h_ + 1) * 2 * 4096]
                    .rearrange("p (r n) -> p r n", n=4096),
                    Wi1_d[h_ * 256:(h_ + 1) * 256, :]
                    .rearrange("(r p) n -> p r n", p=128))
            # Wi2/iwq needed first at t=1 (iw_0 is zeros): load after Wi1
            iwq1 = _load_tiled(nc, cst, iwq1_d, 4, 640, F8E4, "iwq1")
            iwq2 = _load_tiled(nc, cst, iwq2_d, 4, 640, F8E4, "iwq2")
            # Wi2 in halves: t=1's gates_iw pair 0 starts on half 1
            Wi2 = cst.tile([128, 4 * 4096], F8E4, name="Wi2")
            for h_ in range(2):
                nc.sync.dma_start(
                    Wi2[:, h_ * 2 * 4096:(h_ + 1) * 2 * 4096]
                    .rearrange("p (r n) -> p r n", n=4096),
                    Wi2_d[h_ * 256:(h_ + 1) * 256, :]
                    .rearrange("(r p) n -> p r n", p=128))
            Wh2o = _load_tiled(nc, cst, Wh2o_d, 8, 512, BF, "Wh2o")
            Wc2o = _load_tiled(nc, cst, Wc2o_d, 4, 512, BF, "Wc2o")
            WoutS = cst.tile([128, 64 * CW], BF, name="WoutS")
            nc.sync.dma_start(WoutS[:], Wout_d[:])
            # Whh residual (Q2) loads last; steps t < RSKIP run Q1-only
            WhhR = _load_tiled(nc, cst, WhhR_d, 8, 4096, F8E4, "WhhR")
            biasLhs, indic32, bdec = sm["biasLhs"], sm["indic32"], sm["bdec"]
            maskE, padT, Pg, ones32, ident, identS = (
                sm["maskE"], sm["padT"], sm["Pg"], sm["ones32"], sm["ident"],
                sm["identS"])
            ctxh = cst.tile([128, 8 * 128], BF, name="ctxh")
            # decT ring of 2 groups: col kt*256 + (g%2)*128 + (t%4)*32 + b
            decT = cst.tile([128, 4 * 256], BF, name="decT")
            mm = nc.tensor.matmul

            def gblk(gp, bi):
                # gates stored in two bank-sized tiles (separate dependency
                # tracking): A = {g,f} blocks bi 0-15, B = {i,o} blocks 16-31
                if bi < 16:
                    return gp[0][:, bi * 32:(bi + 1) * 32]
                return gp[1][:, (bi - 16) * 32:(bi - 15) * 32]

            DRM = mybir.MatmulPerfMode.DoubleRow
            Wi2v = Wi2[:].rearrange("p (pr two n) -> p pr two n", pr=2, two=2)
            Whhv = Whh[:].rearrange("p (pr two n) -> p pr two n", pr=4, two=2)
            WhhRv = WhhR[:].rearrange("p (pr two n) -> p pr two n", pr=4, two=2)
            iwq1v = iwq1[:].rearrange("p (kt n) -> p kt n", kt=4)
            iwq2v = iwq2[:].rearrange("p (kt n) -> p kt n", kt=4)
            hq1v = hq1[:].rearrange("p (pr two n) -> p pr two n", pr=4, two=2)
            hq2v = hq2[:].rearrange("p (pr two n) -> p pr two n", pr=4, two=2)

            def gates_iw(t):
                _sec(nc, 'gatesiw')
                """allocate gps pair for step t; bias + iw part (no h/ctx dep)"""
                gp = (psg.tile([128, 512], F32, name="gpsA", tag="ga"),
                      psg.tile([128, 512], F32, name="gpsB", tag="gb"))
                mm(gp[0][:], biasLhs[:], indic32[:, 0:512],
                   start=True, stop=False)
                mm(gp[1][:], biasLhs[:], indic32[:, 512:1024],
                   start=True, stop=False)
                if t == 0:
                    # iw_0 is exact zeros by construction: its matmuls are an
                    # identity no-op; skipping them lets Wi2/iwq load after
                    # Wi1 (off the startup critical path)
                    return gp
                for pr in range(2):
                    for bi in range(32):
                        blk = gblk(gp, bi)
                        w1 = Wi2v[:, pr, :, bi * 128:(bi + 1) * 128]
                        for xv in (iwq1v, iwq2v):
                            mm(blk, w1,
                               xv[:, 2 * pr:2 * pr + 2, t * 32:(t + 1) * 32],
                               start=False, stop=False, perf_mode=DRM)
                return gp

            def dec_group(gq):
                _sec(nc, 'dec')
                # Wh2o/Wc2o lhsT tiles streamed from HBM per group
                s0 = (4 * gq) % 8
                g2 = gq % 2
                hv = hist[:].rearrange("p (s r b) -> p s r b", s=8, b=32)
                cv = ctxh[:].rearrange("p (s r b) -> p s r b", s=8, b=32)
                dps = psm.tile([128, 512], F32, name="dps", tag="x")
                for r in range(8):
                    for m in range(4):
                        mm(dps[:, m * 128:(m + 1) * 128],
                           Wh2o[:, r * 512 + m * 128: r * 512 + (m + 1) * 128],
                           hv[:, s0:s0 + 4, r, :],
                           start=(r == 0 and m == 0), stop=False)
                for r in range(4):
                    for m in range(4):
                        mm(dps[:, m * 128:(m + 1) * 128],
                           Wc2o[:, r * 512 + m * 128: r * 512 + (m + 1) * 128],
                           cv[:, s0:s0 + 4, r, :], start=False, stop=False)
                for m in range(4):
                    mm(dps[:, m * 128:(m + 1) * 128], identS[:],
                       iwq1[:, m * 640 + gq * 128: m * 640 + (gq + 1) * 128],
                       start=False, stop=False)
                for m in range(4):
                    mm(dps[:, m * 128:(m + 1) * 128], identS[:],
                       iwq2[:, m * 640 + gq * 128: m * 640 + (gq + 1) * 128],
                       start=False, stop=True)
                for m in range(4):
                    nc.scalar.activation(
                        decT[:, m * 256 + g2 * 128: m * 256 + (g2 + 1) * 128],
                        dps[:, m * 128:(m + 1) * 128], AF.Tanh,
                        bias=bdec[:, m:m + 1])

            def vocab_chunks(gq, cis, fine=False):
                # consecutive chunks paired into one output DMA so the store
                # has >=512B contiguous runs (sub-512B runs pay 2x latency);
                # fine=True (endgame drain) goes chunk-at-a-time so the
                # mm->copy->store pipeline is twice as deep
                _sec(nc, 'vocab')
                g2 = gq % 2
                cis = list(cis)
                i = 0
                while i < len(cis):
                    pair = cis[i:i + 1] if fine else cis[i:i + 2]
                    if len(pair) == 2 and pair[1] != pair[0] + 1:
                        pair = pair[:1]
                    lgv = wkv.tile([128, 2 * CW], BF, name="lgv")
                    vps = psv.tile([128, 2 * CW], F32, name="vps")
                    for j, ci in enumerate(pair):
                        for kt in range(4):
                            mm(vps[:, j * CW:(j + 1) * CW],
                               decT[:, kt * 256 + g2 * 128: kt * 256 + (g2 + 1) * 128],
                               WoutS[:, ci * 4 * CW + kt * CW: ci * 4 * CW + (kt + 1) * CW],
                               start=(kt == 0), stop=(kt == 3))
                    # GPSIMD can't read PSUM on HW; alternate ACT/DVE to
                    # spread the eviction load across both engines
                    if (pair[0] // 2) % 2 == 0:
                        nc.scalar.copy(lgv[:, 0:len(pair) * CW],
                                       vps[:, 0:len(pair) * CW])
                    else:
                        nc.vector.tensor_copy(lgv[:, 0:len(pair) * CW],
                                              vps[:, 0:len(pair) * CW])
                    nc.sync.dma_start(
                        out_d[gq * 128:(gq + 1) * 128,
                              pair[0] * CW: pair[0] * CW + len(pair) * CW],
                        lgv[:, 0:len(pair) * CW])
                    i += len(pair)

            for rep in range(repeat):
                if rep > 0:
                    nc.sync.dma_start(hist[:, 7 * 256:8 * 256], h0_d[:])
                    nc.sync.dma_start(cT[:], c0_d[:])
                gps = None
                for t in range(T):
                    s = (t - 1) % 8
                    w = t % 8
                    hs = lambda kt: hist[:, s * 256 + kt * 32: s * 256 + kt * 32 + 32]
                    # -- u = Wa^T h  (512, 32) as (128, 4x32)
                    _sec(nc, 'u')
                    pu = psm.tile([128, 128], F32, name="pu", tag="x")
                    for rh in range(2):
                        for m in range(4):
                            for r in range(rh * 4, rh * 4 + 4):
                                mm(pu[:, m * 32:(m + 1) * 32],
                                   Wa[:, r * 512 + m * 128: r * 512 + (m + 1) * 128],
                                   hs(r), start=(rh == 0 and m == 0 and r == 0),
                                   stop=(r == 7))
                    u = wk.tile([128, 128], BF, name="u", bufs=2)
                    nc.scalar.copy(u[:], pu[:])
                    # -- scoresT (2048, 32) as (128, 16x32)
                    _sec(nc, 'scoresT')
                    sps = psps.tile([128, 128], F32, name="sps")
                    for kt in range(4):
                        for g in range(4):
                            for m in range(4):
                                mm(sps[:, g * 32 + m * 8: g * 32 + m * 8 + 8],
                                   featsT[:, kt * 2048 + (m * 4 + g) * 128:
                                          kt * 2048 + (m * 4 + g) * 128 + 128],
                                   u[:, kt * 32 + g * 8: kt * 32 + g * 8 + 8],
                                   start=(kt == 0 and g == 0 and m == 0),
                                   stop=(kt == 3))
                    if t == 0:
                        # derive fblk = featsT^T block-wise (PE transpose via
                        # identity rhs, Pool copies PSUM->SBUF); runs in the
                        # startup DMA window, saves a 2MB HBM load
                        _sec(nc, 'transp')
                        for mg in range(16):
                            tps = psv.tile([128, 4 * 128], BF, name="tps",
                                           tag="vps")
                            for kt in range(4):
                                mm(tps[:, kt * 128:(kt + 1) * 128],
                                   featsT[:, kt * 2048 + mg * 128:
                                          kt * 2048 + mg * 128 + 128],
                                   ident[:], is_transpose=True,
                                   start=True, stop=True)
                            nc.vector.tensor_copy(
                                fblk[:, mg * 512:(mg + 1) * 512], tps[:])
                    # -- gates bias+iw part (t=0 only; later steps emit it
                    # at the end of the previous step to fill the LSTM gap)
                    _sec(nc, 'gates0')
                    if gps is None:
                        gps = gates_iw(0)
                    _sec(nc, 'gatesWhh')
                    # -- gates h part: DR 3-term (Q1w*x1, Q2w*x1, Q1w*x2),
                    # pair-phased so the first Whh DMA half serves step 0.
                    # Steps 0-1 skip the Q2w residual so its 4MB DMA can land
                    # off the startup critical path (err contribution tested
                    # in acc_model: negligible).
                    for prh in range(2):
                        for bi in range(32):
                            blk = gblk(gps, bi)
                            for pr in range(prh * 2, prh * 2 + 2):
                                x1 = hq1v[:, pr, :, :]
                                mm(blk, Whhv[:, pr, :, bi * 128:(bi + 1) * 128],
                                   x1, start=False, stop=False, perf_mode=DRM)
                                if t >= RSKIP:
                                    mm(blk, WhhRv[:, pr, :, bi * 128:(bi + 1) * 128],
                                       x1, start=False, stop=False, perf_mode=DRM)
                                mm(blk, Whhv[:, pr, :, bi * 128:(bi + 1) * 128],
                                   hq2v[:, pr, :, :],
                                   start=False, stop=False, perf_mode=DRM)
                    _sec(nc, 'dec+v2')
                    # deferred dec for the previous group
                    if t % 4 == 0 and t > 0:
                        dec_group(t // 4 - 1)
                    # 2 vocab chunks fill the softmax window
                    if t >= 4:
                        _lo, _hi = [(0, 1), (1, 6), (6, 11), (11, 16)][t % 4]
                        _mid = _lo + (1 if t % 4 == 0 else 2)
                        vocab_chunks(t // 4 - 1, range(_lo, _mid))
                    # -- diag extract + softmax (no max-sub; |scores| < 88)
                    _sec(nc, 'softmax')
                    nc.vector.tensor_mul(sps[:], sps[:], maskE[:])
                    sd = wk.tile([128, 16], F32, name="sd", bufs=2)
                    nc.vector.reduce_sum(
                        sd[:], sps[:].rearrange("p (gm c) -> p gm c", c=8), axis=AX.X)
                    nc.vector.tensor_add(sd[:], sd[:], padT[:])
                    ex = wk.tile([128, 16], BF, name="ex", bufs=2)
                    nc.scalar.activation(ex[:], sd[:], AF.Exp)
                    rows = wk.tile([128, 4], F32, name="rows", bufs=2)
                    nc.vector.reduce_sum(
                        rows[:], ex[:].rearrange("p (g m) -> p g m", m=4),
                        axis=AX.X)
                    # dummy: pulls the sigmoid-set table load (1.3us) into the
                    # post-exp window instead of the LSTM critical chain
                    dum = wk.tile([128, 1], F32, name="dum")
                    nc.scalar.activation(dum[:], rows[:, 0:1], AF.Sigmoid)
                    if HEAT_A:
                        heat = psm.tile([128, 32], F32, name="heat", tag="x")
                        for _hk in range(HEAT_A):
                            mm(heat[:], Wa[:, 0:128], Wa[:, 0:32],
                               start=True, stop=True)
                    # unnormalized diag expansion FIRST: aEs = ex * maskE is
                    # the only gate for ctx; the 1/sum normalizer (pS/rS/rbB)
                    # is built while the ctx matmuls run and lands in rbBs for
                    # the eviction stt. pS/rbB live in the sps PSUM ring
                    # (dead after the mask-mul) to avoid a tag-x deadlock.
                    aEs = wk.tile([128, 128], BF, name="aEs", bufs=2)
                    nc.vector.scalar_tensor_tensor(
                        aEs[:].rearrange("p (gm c) -> p gm c", c=8),
                        ex[:].rearrange("p gm -> p gm ()").broadcast_to([128, 16, 8]),
                        1.0,
                        maskE[:].rearrange("p (gm c) -> p gm c", c=8),
                        ALU.mult, ALU.mult)
                    # -- ctxT (512, 32) as (128, 4x32)
                    _sec(nc, 'ctx')
                    cps = psm.tile([128, 128], F32, name="cps", tag="x")
                    for mf in range(4):
                        for g in range(4):
                            for ml in range(4):
                                mm(cps[:, mf * 32 + g * 8: mf * 32 + g * 8 + 8],
                                   fblk[:, (ml * 4 + g) * 512 + mf * 128:
                                        (ml * 4 + g) * 512 + mf * 128 + 128],
                                   aEs[:, g * 32 + ml * 8: g * 32 + ml * 8 + 8],
                                   start=(ml == 0), stop=(ml == 3))
                    # normalizer, hidden behind the ctx matmuls
                    _sec(nc, 'softmax')
                    pS = psps.tile([8, 4], F32, name="pS", tag="sps")
                    mm(pS[:], Pg[:], rows[:], start=True, stop=True)
                    rS = wk.tile([8, 4], F32, name="rS")
                    nc.vector.reciprocal(rS[:], pS[:])
                    # rSd cols (g, c): rS[p8, g] * (p8 == c), b-ordered
                    rSd = wk.tile([8, 32], F32, name="rSd")
                    nc.vector.scalar_tensor_tensor(
                        rSd[:].rearrange("p (g c) -> p g c", c=8),
                        rS[:].rearrange("p g -> p g ()").broadcast_to([8, 4, 8]),
                        128.0,
                        Pg[0:8, 0:8].rearrange("p c -> p () c").broadcast_to([8, 4, 8]),
                        ALU.mult, ALU.mult)
                    rbB = psps.tile([128, 32], F32, name="rbB", tag="sps")
                    mm(rbB[:], ones32[:], rSd[:], start=True, stop=True)
                    # HW: DVE may read only ONE non-scalar PSUM input; the
                    # norm stt also reads cps, so rbB must bounce via SBUF
                    rbBs = wk.tile([128, 32], F32, name="rbBs")
                    nc.vector.tensor_copy(rbBs[:], rbB[:])
                    _sec(nc, 'ctx')
                    # normalize while evicting: ctxh = cps * rbBs
                    nc.vector.scalar_tensor_tensor(
                        ctxh[:, w * 128:(w + 1) * 128]
                        .rearrange("p (k c) -> p k c", c=32),
                        cps[:].rearrange("p (k c) -> p k c", c=32), 1.0,
                        rbBs[:].rearrange("p c -> p () c").broadcast_to([128, 4, 32]),
                        ALU.mult, ALU.mult)
                    # -- gates ctx part: bank0 gates (g,f) first, their
                    # activations + c-mul overlap the bank1 (i,o) matmuls
                    # (bank-aligned so no PSUM-bank ping-pong)
                    _sec(nc, 'Wi1+act')
                    gG, gF, gI, gO = (gps[0][:, 0:256], gps[0][:, 256:512],
                                      gps[1][:, 0:256], gps[1][:, 256:512])
                    tGs = wk.tile([128, 256], BF, name="tGs", bufs=2)
                    def wi1_sec(gsec):
                        for bi in range(gsec * 8, gsec * 8 + 8):
                            blk = gblk(gps, bi)
                            for kt in range(4):
                                mm(blk, Wi1[:, kt * 4096 + bi * 128: kt * 4096 + (bi + 1) * 128],
                                   ctxh[:, w * 128 + kt * 32: w * 128 + (kt + 1) * 32],
                                   start=False, stop=(kt == 3))
                    wi1_sec(0)
                    wi1_sec(1)
                    wi1_sec(2)
                    wi1_sec(3)
                    gps_next = gates_iw(t + 1) if t + 1 < T else None
                    _sec(nc, 'Wi1+act')
                    nc.scalar.activation(tGs[:], gG, AF.Tanh, scale=1.0 / (WSCALE * XS))
                    sF = wk.tile([128, 256], BF, name="sF", bufs=2)
                    nc.scalar.activation(sF[:], gF, AF.Sigmoid, scale=1.0 / (WSCALE * XS))
                    nc.vector.tensor_mul(cT[:], cT[:], sF[:])
                    # gate activations land in SBUF bf16: frees the gps PSUM
                    # bank early and enables 2x DVE on the i*g / o*tanh muls
                    gIs = wk.tile([128, 256], BF, name="gIs", bufs=2)
                    nc.scalar.activation(gIs[:], gI, AF.Sigmoid, scale=1.0 / (WSCALE * XS))
                    gOs = wk.tile([128, 256], BF, name="gOs", bufs=2)
                    nc.scalar.activation(gOs[:], gO, AF.Sigmoid, scale=1.0 / (WSCALE * XS))
                    # -- LSTM-gap fillers: next step's dep-free gate matmuls
                    # first (no DMA dependency), then 2 more vocab chunks
                    _sec(nc, 'giw+v2b')
                    # -- p-state heater: dep-free dummy matmuls keep the PE
                    # clock streak alive through the LSTM tail (a broken
                    # streak costs ~3us of half-speed ramp-up)
                    if HEAT_B and t >= 15:
                        heat2 = psm.tile([128, 32], F32, name="heat", tag="x")
                        for _hk in range(HEAT_B):
                            mm(heat2[:], Wa[:, 0:128], Wa[:, 0:32],
                               start=True, stop=True)
                    # -- LSTM elementwise tail (activations emitted above)
                    _sec(nc, 'lstmtail')
                    nc.vector.tensor_mul(gIs[:], gIs[:], tGs[:])
                    nc.vector.tensor_add(cT[:], cT[:], gIs[:])
                    tCs = wk.tile([128, 256], BF, name="tCs", bufs=2)
                    nc.scalar.activation(tCs[:, 0:128], cT[:, 0:128], AF.Tanh)
                    nc.vector.tensor_mul(hist[:, w * 256: w * 256 + 128],
                                         gOs[:, 0:128], tCs[:, 0:128])
                    nc.scalar.activation(tCs[:, 128:256], cT[:, 128:256], AF.Tanh)
                    nc.vector.tensor_mul(hist[:, w * 256 + 128:(w + 1) * 256],
                                         gOs[:, 128:256], tCs[:, 128:256])
                    # e4m3 pair quantization of h for next step's DR gates
                    hsl = hist[:, w * 256:(w + 1) * 256]
                    nc.scalar.activation(hq1[:], hsl, AF.Copy, scale=XS)
                    nc.vector.scalar_tensor_tensor(
                        hq2[:], hsl, XS, hq1[:], ALU.mult, ALU.subtract)
                    # tail-slot vocab emitted AFTER the tail ops: their PE
                    # matmuls still fill this window, but the PSUM->SBUF
                    # copies now queue BEHIND the critical tail ACT/DVE ops
                    # instead of blocking them (in-order engine queues)
                    if t >= 4:
                        vocab_chunks(t // 4 - 1, range(_mid, _hi))
                    gps = gps_next
                # tail: last dec group + its vocab
                dec_group(4)
                vocab_chunks(4, range(16))
                if dbg:
                    nc.sync.dma_start(dbg_hist[:], hist[:])
                    nc.sync.dma_start(dbg_ctxh[:], ctxh[:])
                    nc.sync.dma_start(dbg_cT[:], cT[:])
    nc.finalize()
    return nc


def kernel(**inputs) -> np.ndarray:
    if "nc" not in _BUILT:
        _BUILT["nc"] = build()
    nc = _BUILT["nc"]
    in_maps = host_prep(inputs)
    res = run_bass_kernel_spmd(nc, in_maps, core_ids=list(range(NC)))
    full = np.concatenate(
        [np.asarray(res.results[k]["out"]) for k in range(NC)], axis=1)
    # (640, 32000) bf16, row t*32+b -> (B, T, V) f32
    out = np.ascontiguousarray(
        full.reshape(T, B, V).transpose(1, 0, 2)).astype(np.float32)
    b_out = np.asarray(inputs["b_out"], np.float32)
    if np.any(b_out):
        out += b_out[None, None, :]
    return out



# revision 94
# speedup vs baseline: 1.0199x; 1.0050x over previous
"""Trainium2 Bass kernel for nn_AttnDecoderRNN (B=32,T=20,L=49,F=512,H=1024,V=32000).

Zero-collective design across 8 NeuronCores:
- The attention-LSTM recurrence is fully REPLICATED on every core (tensor-
  parallel splits need a per-step AllGather of h, which dominates cost).
- Only the vocab projection is tensor-parallel: core k owns W_out rows
  [4000k, 4000(k+1)) and emits a (640, 4000) bf16 logit shard; the host
  concatenation is the all-gather. b_out is added host-side (skipped if 0).

Per-core schedule (all layouts feature-on-partition):
- scoresT attention: sps[(l,b), b'] = feats . (Wa^T h), diagonal extracted
  with a one-hot mask + grouped reduce; softmax runs in the (l%4 x b, l//4)
  layout with tiny PE sum/broadcast matmuls; normalize+expand fused into one
  scalar_tensor_tensor.
- gates: one PSUM accumulation (bias matmul + iw + h + ctx parts); the
  LSTM elementwise runs mostly in place on PSUM; a dummy sigmoid after the
  softmax Exp prefetches the ACT table set off the critical chain.
- dec groups (4 steps) with Wh2o/Wc2o streamed from HBM; vocab chunks are
  interleaved into the step loop (W_out streamed) to keep the PE fed
  through the softmax/LSTM dependency-chain windows.
"""
import sys

sys.path.insert(0, "/opt/trn_rl_repo")
import numpy as np
import ml_dtypes

import concourse.bass as bass
import concourse.mybir as mybir
import concourse.tile as tile
from concourse import bacc
from concourse.bass_utils import run_bass_kernel_spmd

B, T, L, F, H, V = 32, 20, 49, 512, 1024, 32000
LP = 64
MB = 13            # l-blocks kept (l < 4*MB covers L=49; blocks 13-15 are all-pad)
NC = 8
VS = V // NC      # 4000
CW = 250          # vocab chunk width (16 chunks per group)
BF = mybir.dt.bfloat16
F32 = mybir.dt.float32
NBF = ml_dtypes.bfloat16
NF8 = ml_dtypes.float8_e3m4
NE4 = ml_dtypes.float8_e4m3
WSCALE = 512.0         # gate weight pre-scale (e4m3 normal range)
XS = 32.0              # gate rhs (h/ctx/iw) pre-scale for e4m3
GPERM = [2, 1, 0, 3]   # gate storage g,f,i,o: bank0={g,f} early, bank1={i,o} late
HEAT_A = 0            # heater matmuls filling the softmax window
HEAT_B = 0            # heater matmuls filling the LSTM-tail window
RSKIP = 6              # steps running without the Whh Q2 residual (DMA window)
# DoubleRow fp8 gates: weights pair-quantized (Q1+Q2) where listed 3,
# single-quantized (Q1 only) where 2; rhs always pair-quantized. Each
# DR matmul covers 2 k-tiles at 0.5 cycles/row. Wi1 (value 0) stays
# plain e3m4 at x128 with a bf16 ctx rhs scaled x128 (rbBs carries the
# 128; Wc2o is pre-divided by 128 to compensate on the dec path) --
# its pair-residual would not fit SBUF.
PLAN = {"Whh": 3, "Wi1": 0, "Wi2": 2}


def _qpair(x):
    q1 = x.astype(NE4)
    q2 = (x - q1.astype(np.float32)).astype(NE4)
    return q1, q2

_BUILT = {}
SECTIONS = []


def _sec(nc, label):
    SECTIONS.append((int(nc.get_next_instruction_name().split('-')[1]), label))


def host_prep(inputs):
    f32 = lambda x: np.asarray(x, np.float32)
    feats = f32(inputs["features"])                    # (B, F, L)
    cap = np.asarray(inputs["captions"])
    emb = np.asarray(inputs["embed_table"])
    fpad = np.zeros((LP, B, F), np.float32)
    fpad[:L] = feats.transpose(2, 0, 1)
    # 8-wide score layout: partition p8 = (l%16)*8 + b%8, col blocks (m=l//16,
    # g=b//8); featsT8 (512, [kt is rows][m][g][p8])
    featsT = np.ascontiguousarray(
        fpad.reshape(4, 16, 4, 8, 512).transpose(4, 0, 2, 1, 3)
        .reshape(512, 2048))
    h0 = np.tanh(feats.mean(axis=2) @ f32(inputs["W_init"]).T + f32(inputs["b_init"]))
    h0T = h0.T                                         # (1024, 32)
    h0slot = np.ascontiguousarray(
        h0T.reshape(8, 128, B).transpose(1, 0, 2)).reshape(128, 256)
    h0q1, h0q2 = _qpair(h0slot.astype(NBF).astype(np.float32) * XS)
    e = f32(emb[cap])
    iw = np.concatenate([np.zeros((B, 1, F), np.float32), e[:, :-1]], axis=1)
    iwT = np.ascontiguousarray(iw.transpose(2, 1, 0)).reshape(F, T * B)
    iwq1, iwq2 = _qpair(iwT.astype(NBF).astype(np.float32) * XS)
    Wih = f32(inputs["W_ih"])
    # permute gate blocks: storage pos p holds original gate GPERM[p]
    def gperm_cols(W):                                 # W (K, 4096) -> permuted
        return np.ascontiguousarray(
            W.reshape(-1, 4, 1024)[:, GPERM].reshape(-1, 4096))
    Whh = gperm_cols(f32(inputs["W_hh"]).T)            # (1024, 4096)
    Wi1 = gperm_cols(Wih[:, :F].T)                     # (512, 4096)
    Wi2 = gperm_cols(Wih[:, F:].T)                     # (512, 4096)
    bg = (f32(inputs["b_ih"]) + f32(inputs["b_hh"])).reshape(4, 8, 128)[GPERM]
    indic32 = np.zeros((32, 1024), np.float32)
    for k in range(32):
        indic32[k, k * 32:(k + 1) * 32] = 1.0
    # maskE8 cols (g, m, c8): diag selector c == p8%8
    maskE = np.zeros((128, 128), np.float32)
    for p in range(128):
        for g in range(4):
            for m in range(4):
                maskE[p, g * 32 + m * 8 + (p % 8)] = 1.0
    # padT8 cols (g, m): pad where l = 16m + p8//8 >= L
    padT = np.zeros((128, 16), np.float32)
    for p in range(128):
        for g in range(4):
            for m in range(4):
                if 16 * m + p // 8 >= L:
                    padT[p, g * 4 + m] = -30000.0
    Pg = np.zeros((128, 8), np.float32)
    for p in range(128):
        Pg[p, p % 8] = 1.0
    bdec = (f32(inputs["b_h2o"]) + f32(inputs["b_c2o"])).reshape(4, 128).T
    shared = {
        "featsT": featsT.astype(NBF),
        "h0slot": h0slot.astype(NBF), "c0": h0slot.astype(np.float32),
        "h0q1": h0q1, "h0q2": h0q2,
        "iwq1": iwq1, "iwq2": iwq2,
        "ones32": np.ones((8, 128), np.float32),
        "Wa": f32(inputs["Wa"]).astype(NBF),           # (1024, 512) lhsT
        # gate weights pair-quantized e4m3 at xWSCALE (DoubleRow matmuls);
        # rhs x-side pair-quantized at xXS; activation scale 1/(WSCALE*XS)
        "biasLhs": (bg.reshape(32, 128) * WSCALE * XS).astype(NBF),
        "indic32": indic32.astype(NBF),
        "Wh2o": f32(inputs["W_h2o"]).T.astype(NBF),    # (1024, 512)
        # ctxh carries ctx*128 (for the e3m4 Wi1 path); Wc2o compensates
        "Wc2o": (f32(inputs["W_c2o"]).T / 128.0).astype(NBF),  # (512, 512)
        "bdec": np.ascontiguousarray(bdec),
        "maskE": maskE.astype(NBF), "padT": padT,
        "Pg": Pg,
        "ident": np.eye(128, dtype=np.float32).astype(NBF),
        "identS": (np.eye(128, dtype=np.float32) / XS).astype(NBF),
    }
    for nm, W in (("Whh", Whh), ("Wi1", Wi1), ("Wi2", Wi2)):
        if PLAN[nm] == 0:
            shared[nm] = (W * 128.0).astype(NF8)
            continue
        q1, q2 = _qpair(W * WSCALE)
        shared[nm] = q1
        if PLAN[nm] == 3:
            shared[nm + "R"] = q2
    WoutT = f32(inputs["W_out"]).T                     # (512, 32000)
    in_maps = []
    for k in range(NC):
        m = dict(shared)
        # partition-major chunk layout: row p holds [ci][kt][n] so each
        # chunk DMA is one contiguous (128, 1000-elem) slice (>=512B runs)
        m["Wout"] = np.ascontiguousarray(
            WoutT[:, VS * k:VS * (k + 1)].reshape(4, 128, 16, CW)
            .transpose(1, 2, 0, 3).reshape(128, 64 * CW)).astype(NBF)
        in_maps.append(m)
    return in_maps


def _load_tiled(nc, pool, dram, KT, N, dtype, name):
    """dram (KT*128, N) -> sbuf (128, KT*N), col block kt holds rows kt*128.."""
    t = pool.tile([128, KT * N], dtype, name=name)
    src = dram[:].rearrange("(r p) n -> p r n", p=128)
    dst = t[:].rearrange("p (r n) -> p r n", n=N)
    nc.sync.dma_start(dst, src)
    return t


def build(repeat=1, dbg=False):
    nc = bacc.Bacc("TRN2", target_bir_lowering=False, debug=False, num_devices=NC)
    di = lambda nm, sh, dt: nc.dram_tensor(nm, list(sh), dt, kind="ExternalInput")
    featsT_d = di("featsT", (512, 2048), BF)
    ones32_d = di("ones32", (8, 128), F32)
    h0_d = di("h0slot", (128, 256), BF)
    c0_d = di("c0", (128, 256), F32)
    F8E4 = mybir.dt.float8e4
    h0q1_d = di("h0q1", (128, 256), F8E4)
    h0q2_d = di("h0q2", (128, 256), F8E4)
    iwq1_d = di("iwq1", (512, 640), F8E4)
    iwq2_d = di("iwq2", (512, 640), F8E4)
    Wa_d = di("Wa", (1024, 512), BF)
    F8 = mybir.dt.float8e3
    Whh_d = di("Whh", (1024, 4096), F8E4)
    Wi1_d = di("Wi1", (512, 4096), F8)
    Wi2_d = di("Wi2", (512, 4096), F8E4)
    WhhR_d = di("WhhR", (1024, 4096), F8E4)
    biasLhs_d = di("biasLhs", (32, 128), BF)
    indic32_d = di("indic32", (32, 1024), BF)
    Wh2o_d = di("Wh2o", (1024, 512), BF)
    Wc2o_d = di("Wc2o", (512, 512), BF)
    bdec_d = di("bdec", (128, 4), F32)
    Wout_d = di("Wout", (128, 64 * CW), BF)
    maskE_d = di("maskE", (128, 128), BF)
    padT_d = di("padT", (128, 16), F32)
    Pg_d = di("Pg", (128, 8), F32)

    ident_d = di("ident", (128, 128), BF)
    identS_d = di("identS", (128, 128), BF)
    out_d = nc.dram_tensor("out", [T * B, VS], BF, kind="ExternalOutput")
    if dbg:
        dbg_hist = nc.dram_tensor("dbg_hist", [128, 8 * 256], BF, kind="ExternalOutput")
        dbg_ctxh = nc.dram_tensor("dbg_ctxh", [128, 8 * 128], BF, kind="ExternalOutput")
        dbg_cT = nc.dram_tensor("dbg_cT", [128, 256], F32, kind="ExternalOutput")

    AF = mybir.ActivationFunctionType
    AX = mybir.AxisListType
    ALU = mybir.AluOpType
    with tile.TileContext(nc) as tc:
        with tc.tile_pool(name="cst", bufs=1) as cst, \
             tc.tile_pool(name="wk", bufs=3) as wk, \
             tc.tile_pool(name="wkd", bufs=3) as wkd, \
             tc.tile_pool(name="wkv", bufs=2) as wkv, \
             tc.tile_pool(name="psg", bufs=2, space="PSUM") as psg, \
             tc.tile_pool(name="psv", bufs=2, space="PSUM") as psv, \
             tc.tile_pool(name="psps", bufs=1, space="PSUM") as psps, \
             tc.tile_pool(name="psm", bufs=1, space="PSUM") as psm:
            # ---- persistent SBUF ----
            hist = cst.tile([128, 8 * 256], BF, name="hist")
            cT = cst.tile([128, 256], F32, name="cT")
            # Wa first (u needs it + h0; the half-split lets the r-phased u
            # accumulation start on half 1), then the tiny h0/c0/hq seeds
            Wa = cst.tile([128, 8 * 512], BF, name="Wa")
            for h_ in range(2):
                nc.sync.dma_start(
                    Wa[:, h_ * 4 * 512:(h_ + 1) * 4 * 512]
                    .rearrange("p (r n) -> p r n", n=512),
                    Wa_d[h_ * 512:(h_ + 1) * 512, :]
                    .rearrange("(r p) n -> p r n", p=128))
            nc.sync.dma_start(hist[:, 7 * 256:8 * 256], h0_d[:])
            nc.sync.dma_start(cT[:], c0_d[:])
            # per-step e4m3 pair quantizations of h (persistent, rewritten
            # each step; seeded with h0 pairs -- must land before step-0 gates)
            hq1 = cst.tile([128, 256], F8E4, name="hq1")
            hq2 = cst.tile([128, 256], F8E4, name="hq2")
            nc.sync.dma_start(hq1[:], h0q1_d[:])
            nc.sync.dma_start(hq2[:], h0q2_d[:])
            featsT = cst.tile([128, 4 * 2048], BF, name="featsT")
            for kt_ in range(4):
                nc.sync.dma_start(
                    featsT[:, kt_ * 2048:(kt_ + 1) * 2048],
                    featsT_d[kt_ * 128:(kt_ + 1) * 128, :])
            small = [("biasLhs", biasLhs_d, [32, 128], BF),
                     ("indic32", indic32_d, [32, 1024], BF),
                     ("maskE", maskE_d, [128, 128], BF),
                     ("padT", padT_d, [128, 16], F32),
                     ("Pg", Pg_d, [128, 8], F32),
                     ("ones32", ones32_d, [8, 128], F32),
                     ("ident", ident_d, [128, 128], BF)]
            sm = {}
            for nm, d, sh, dt in small:
                sm[nm] = cst.tile(sh, dt, name=nm)
                nc.sync.dma_start(sm[nm][:], d[:])
            # Whh split into halves: step-0 gates on kt 0-3 start ~6us sooner
            Whh = cst.tile([128, 8 * 4096], F8E4, name="Whh")
            for h_ in range(2):
                nc.sync.dma_start(
                    Whh[:, h_ * 4 * 4096:(h_ + 1) * 4 * 4096]
                    .rearrange("p (r n) -> p r n", n=4096),
                    Whh_d[h_ * 512:(h_ + 1) * 512, :]
                    .rearrange("(r p) n -> p r n", p=128))
            # fblk derived on-chip from featsT (PE transposes + DVE copies)
            # instead of a second 2MB DMA of the same data
            fblk = cst.tile([128, 16 * 512], BF, name="fblk")
            # Wi1 split into kt-halves: step-0 wi1 phase 0 starts ~3us sooner
            Wi1 = cst.tile([128, 4 * 4096], F8, name="Wi1")
            for h_ in range(2):
                nc.sync.dma_start(
                    Wi1[:, h_ * 2 * 4096:(h_ + 1) * 2 * 4096]
                    .rearrange("p (r n) -> p r n", n=4096),
                    Wi1_d[h_ * 256:(h_ + 1) * 256, :]
                    .rearrange("(r p) n -> p r n", p=128))
            # Wi2/iwq needed first at t=1 (iw_0 is zeros): load after Wi1
            iwq1 = _load_tiled(nc, cst, iwq1_d, 4, 640, F8E4, "iwq1")
            iwq2 = _load_tiled(nc, cst, iwq2_d, 4, 640, F8E4, "iwq2")
            # Wi2 in halves: t=1's gates_iw pair 0 starts on half 1
            Wi2 = cst.tile([128, 4 * 4096], F8E4, name="Wi2")
            for h_ in range(2):
                nc.sync.dma_start(
                    Wi2[:, h_ * 2 * 4096:(h_ + 1) * 2 * 4096]
                    .rearrange("p (r n) -> p r n", n=4096),
                    Wi2_d[h_ * 256:(h_ + 1) * 256, :]
                    .rearrange("(r p) n -> p r n", p=128))
            for nm, d, sh, dt in (("bdec", bdec_d, [128, 4], F32),
                                  ("identS", identS_d, [128, 128], BF)):
                sm[nm] = cst.tile(sh, dt, name=nm)
                nc.sync.dma_start(sm[nm][:], d[:])
            Wh2o = _load_tiled(nc, cst, Wh2o_d, 8, 512, BF, "Wh2o")
            Wc2o = _load_tiled(nc, cst, Wc2o_d, 4, 512, BF, "Wc2o")
            WoutS = cst.tile([128, 64 * CW], BF, name="WoutS")
            nc.sync.dma_start(WoutS[:], Wout_d[:])
            # Whh residual (Q2) loads last; steps t < RSKIP run Q1-only
            WhhR = _load_tiled(nc, cst, WhhR_d, 8, 4096, F8E4, "WhhR")
            biasLhs, indic32, bdec = sm["biasLhs"], sm["indic32"], sm["bdec"]
            maskE, padT, Pg, ones32, ident, identS = (
                sm["maskE"], sm["padT"], sm["Pg"], sm["ones32"], sm["ident"],
                sm["identS"])
            ctxh = cst.tile([128, 8 * 128], BF, name="ctxh")
            # decT ring of 2 groups: col kt*256 + (g%2)*128 + (t%4)*32 + b
            decT = cst.tile([128, 4 * 256], BF, name="decT")
            mm = nc.tensor.matmul

            def gblk(gp, bi):
                # gates stored in two bank-sized tiles (separate dependency
                # tracking): A = {g,f} blocks bi 0-15, B = {i,o} blocks 16-31
                if bi < 16:
                    return gp[0][:, bi * 32:(bi + 1) * 32]
                return gp[1][:, (bi - 16) * 32:(bi - 15) * 32]

            DRM = mybir.MatmulPerfMode.DoubleRow
            Wi2v = Wi2[:].rearrange("p (pr two n) -> p pr two n", pr=2, two=2)
            Whhv = Whh[:].rearrange("p (pr two n) -> p pr two n", pr=4, two=2)
            WhhRv = WhhR[:].rearrange("p (pr two n) -> p pr two n", pr=4, two=2)
            iwq1v = iwq1[:].rearrange("p (kt n) -> p kt n", kt=4)
            iwq2v = iwq2[:].rearrange("p (kt n) -> p kt n", kt=4)
            hq1v = hq1[:].rearrange("p (pr two n) -> p pr two n", pr=4, two=2)
            hq2v = hq2[:].rearrange("p (pr two n) -> p pr two n", pr=4, two=2)

            def gates_iw(t):
                _sec(nc, 'gatesiw')
                """allocate gps pair for step t; bias + iw part (no h/ctx dep)"""
                gp = (psg.tile([128, 512], F32, name="gpsA", tag="ga"),
                      psg.tile([128, 512], F32, name="gpsB", tag="gb"))
                mm(gp[0][:], biasLhs[:], indic32[:, 0:512],
                   start=True, stop=False)
                mm(gp[1][:], biasLhs[:], indic32[:, 512:1024],
                   start=True, stop=False)
                if t == 0:
                    # iw_0 is exact zeros by construction: its matmuls are an
                    # identity no-op; skipping them lets Wi2/iwq load after
                    # Wi1 (off the startup critical path)
                    return gp
                for pr in range(2):
                    for bi in range(32):
                        blk = gblk(gp, bi)
                        w1 = Wi2v[:, pr, :, bi * 128:(bi + 1) * 128]
                        for xv in (iwq1v, iwq2v):
                            mm(blk, w1,
                               xv[:, 2 * pr:2 * pr + 2, t * 32:(t + 1) * 32],
                               start=False, stop=False, perf_mode=DRM)
                return gp

            def dec_group(gq):
                _sec(nc, 'dec')
                # Wh2o/Wc2o lhsT tiles streamed from HBM per group
                s0 = (4 * gq) % 8
                g2 = gq % 2
                hv = hist[:].rearrange("p (s r b) -> p s r b", s=8, b=32)
                cv = ctxh[:].rearrange("p (s r b) -> p s r b", s=8, b=32)
                dps = psm.tile([128, 512], F32, name="dps", tag="x")
                for r in range(8):
                    for m in range(4):
                        mm(dps[:, m * 128:(m + 1) * 128],
                           Wh2o[:, r * 512 + m * 128: r * 512 + (m + 1) * 128],
                           hv[:, s0:s0 + 4, r, :],
                           start=(r == 0 and m == 0), stop=False)
                for r in range(4):
                    for m in range(4):
                        mm(dps[:, m * 128:(m + 1) * 128],
                           Wc2o[:, r * 512 + m * 128: r * 512 + (m + 1) * 128],
                           cv[:, s0:s0 + 4, r, :], start=False, stop=False)
                for m in range(4):
                    mm(dps[:, m * 128:(m + 1) * 128], identS[:],
                       iwq1[:, m * 640 + gq * 128: m * 640 + (gq + 1) * 128],
                       start=False, stop=False)
                for m in range(4):
                    mm(dps[:, m * 128:(m + 1) * 128], identS[:],
                       iwq2[:, m * 640 + gq * 128: m * 640 + (gq + 1) * 128],
                       start=False, stop=True)
                for m in range(4):
                    nc.scalar.activation(
                        decT[:, m * 256 + g2 * 128: m * 256 + (g2 + 1) * 128],
                        dps[:, m * 128:(m + 1) * 128], AF.Tanh,
                        bias=bdec[:, m:m + 1])

            def vocab_chunks(gq, cis, fine=False):
                # consecutive chunks paired into one output DMA so the store
                # has >=512B contiguous runs (sub-512B runs pay 2x latency);
                # fine=True (endgame drain) goes chunk-at-a-time so the
                # mm->copy->store pipeline is twice as deep
                _sec(nc, 'vocab')
                g2 = gq % 2
                cis = list(cis)
                i = 0
                while i < len(cis):
                    pair = cis[i:i + 1] if fine else cis[i:i + 2]
                    if len(pair) == 2 and pair[1] != pair[0] + 1:
                        pair = pair[:1]
                    lgv = wkv.tile([128, 2 * CW], BF, name="lgv")
                    vps = psv.tile([128, 2 * CW], F32, name="vps")
                    for j, ci in enumerate(pair):
                        for kt in range(4):
                            mm(vps[:, j * CW:(j + 1) * CW],
                               decT[:, kt * 256 + g2 * 128: kt * 256 + (g2 + 1) * 128],
                               WoutS[:, ci * 4 * CW + kt * CW: ci * 4 * CW + (kt + 1) * CW],
                               start=(kt == 0), stop=(kt == 3))
                    # GPSIMD can't read PSUM on HW; alternate ACT/DVE to
                    # spread the eviction load across both engines
                    if (pair[0] // 2) % 2 == 0:
                        nc.scalar.copy(lgv[:, 0:len(pair) * CW],
                                       vps[:, 0:len(pair) * CW])
                    else:
                        nc.vector.tensor_copy(lgv[:, 0:len(pair) * CW],
                                              vps[:, 0:len(pair) * CW])
                    nc.sync.dma_start(
                        out_d[gq * 128:(gq + 1) * 128,
                              pair[0] * CW: pair[0] * CW + len(pair) * CW],
                        lgv[:, 0:len(pair) * CW])
                    i += len(pair)

            for rep in range(repeat):
                if rep > 0:
                    nc.sync.dma_start(hist[:, 7 * 256:8 * 256], h0_d[:])
                    nc.sync.dma_start(cT[:], c0_d[:])
                gps = None
                for t in range(T):
                    s = (t - 1) % 8
                    w = t % 8
                    hs = lambda kt: hist[:, s * 256 + kt * 32: s * 256 + kt * 32 + 32]
                    # -- u = Wa^T h  (512, 32) as (128, 4x32)
                    _sec(nc, 'u')
                    pu = psm.tile([128, 128], F32, name="pu", tag="x")
                    for rh in range(2):
                        for m in range(4):
                            for r in range(rh * 4, rh * 4 + 4):
                                mm(pu[:, m * 32:(m + 1) * 32],
                                   Wa[:, r * 512 + m * 128: r * 512 + (m + 1) * 128],
                                   hs(r), start=(rh == 0 and m == 0 and r == 0),
                                   stop=(r == 7))
                    u = wk.tile([128, 128], BF, name="u", bufs=2)
                    nc.scalar.copy(u[:], pu[:])
                    # -- scoresT (2048, 32) as (128, 16x32)
                    _sec(nc, 'scoresT')
                    sps = psps.tile([128, 128], F32, name="sps")
                    for kt in range(4):
                        for g in range(4):
                            for m in range(4):
                                mm(sps[:, g * 32 + m * 8: g * 32 + m * 8 + 8],
                                   featsT[:, kt * 2048 + (m * 4 + g) * 128:
                                          kt * 2048 + (m * 4 + g) * 128 + 128],
                                   u[:, kt * 32 + g * 8: kt * 32 + g * 8 + 8],
                                   start=(kt == 0 and g == 0 and m == 0),
                                   stop=(kt == 3))
                    if t == 0:
                        # derive fblk = featsT^T block-wise (PE transpose via
                        # identity rhs, Pool copies PSUM->SBUF); runs in the
                        # startup DMA window, saves a 2MB HBM load
                        _sec(nc, 'transp')
                        for mg in range(16):
                            tps = psv.tile([128, 4 * 128], BF, name="tps",
                                           tag="vps")
                            for kt in range(4):
                                mm(tps[:, kt * 128:(kt + 1) * 128],
                                   featsT[:, kt * 2048 + mg * 128:
                                          kt * 2048 + mg * 128 + 128],
                                   ident[:], is_transpose=True,
                                   start=True, stop=True)
                            nc.vector.tensor_copy(
                                fblk[:, mg * 512:(mg + 1) * 512], tps[:])
                    # -- gates bias+iw part (t=0 only; later steps emit it
                    # at the end of the previous step to fill the LSTM gap)
                    _sec(nc, 'gates0')
                    if gps is None:
                        gps = gates_iw(0)
                    _sec(nc, 'gatesWhh')
                    # -- gates h part: DR 3-term (Q1w*x1, Q2w*x1, Q1w*x2),
                    # pair-phased so the first Whh DMA half serves step 0.
                    # Steps 0-1 skip the Q2w residual so its 4MB DMA can land
                    # off the startup critical path (err contribution tested
                    # in acc_model: negligible).
                    for prh in range(2):
                        for bi in range(32):
                            blk = gblk(gps, bi)
                            for pr in range(prh * 2, prh * 2 + 2):
                                x1 = hq1v[:, pr, :, :]
                                mm(blk, Whhv[:, pr, :, bi * 128:(bi + 1) * 128],
                                   x1, start=False, stop=False, perf_mode=DRM)
                                if t >= RSKIP:
                                    mm(blk, WhhRv[:, pr, :, bi * 128:(bi + 1) * 128],
                                       x1, start=False, stop=False, perf_mode=DRM)
                                mm(blk, Whhv[:, pr, :, bi * 128:(bi + 1) * 128],
                                   hq2v[:, pr, :, :],
                                   start=False, stop=False, perf_mode=DRM)
                    _sec(nc, 'dec+v2')
                    # deferred dec for the previous group
                    if t % 4 == 0 and t > 0:
                        dec_group(t // 4 - 1)
                    # 2 vocab chunks fill the softmax window
                    if t >= 4:
                        _lo, _hi = [(0, 1), (1, 6), (6, 11), (11, 16)][t % 4]
                        _mid = _lo + (1 if t % 4 == 0 else 2)
                        vocab_chunks(t // 4 - 1, range(_lo, _mid))
                    # -- diag extract + softmax (no max-sub; |scores| < 88)
                    _sec(nc, 'softmax')
                    nc.vector.tensor_mul(sps[:], sps[:], maskE[:])
                    sd = wk.tile([128, 16], F32, name="sd", bufs=2)
                    nc.vector.reduce_sum(
                        sd[:], sps[:].rearrange("p (gm c) -> p gm c", c=8), axis=AX.X)
                    nc.vector.tensor_add(sd[:], sd[:], padT[:])
                    ex = wk.tile([128, 16], BF, name="ex", bufs=2)
                    nc.scalar.activation(ex[:], sd[:], AF.Exp)
                    rows = wk.tile([128, 4], F32, name="rows", bufs=2)
                    nc.vector.reduce_sum(
                        rows[:], ex[:].rearrange("p (g m) -> p g m", m=4),
                        axis=AX.X)
                    # dummy: pulls the sigmoid-set table load (1.3us) into the
                    # post-exp window instead of the LSTM critical chain
                    dum = wk.tile([128, 1], F32, name="dum")
                    nc.scalar.activation(dum[:], rows[:, 0:1], AF.Sigmoid)
                    if HEAT_A:
                        heat = psm.tile([128, 32], F32, name="heat", tag="x")
                        for _hk in range(HEAT_A):
                            mm(heat[:], Wa[:, 0:128], Wa[:, 0:32],
                               start=True, stop=True)
                    # unnormalized diag expansion FIRST: aEs = ex * maskE is
                    # the only gate for ctx; the 1/sum normalizer (pS/rS/rbB)
                    # is built while the ctx matmuls run and lands in rbBs for
                    # the eviction stt. pS/rbB live in the sps PSUM ring
                    # (dead after the mask-mul) to avoid a tag-x deadlock.
                    aEs = wk.tile([128, 128], BF, name="aEs", bufs=2)
                    nc.vector.scalar_tensor_tensor(
                        aEs[:].rearrange("p (gm c) -> p gm c", c=8),
                        ex[:].rearrange("p gm -> p gm ()").broadcast_to([128, 16, 8]),
                        1.0,
                        maskE[:].rearrange("p (gm c) -> p gm c", c=8),
                        ALU.mult, ALU.mult)
                    # -- ctxT (512, 32) as (128, 4x32)
                    _sec(nc, 'ctx')
                    cps = psm.tile([128, 128], F32, name="cps", tag="x")
                    for mf in range(4):
                        for g in range(4):
                            for ml in range(4):
                                mm(cps[:, mf * 32 + g * 8: mf * 32 + g * 8 + 8],
                                   fblk[:, (ml * 4 + g) * 512 + mf * 128:
                                        (ml * 4 + g) * 512 + mf * 128 + 128],
                                   aEs[:, g * 32 + ml * 8: g * 32 + ml * 8 + 8],
                                   start=(ml == 0), stop=(ml == 3))
                    # normalizer, hidden behind the ctx matmuls
                    _sec(nc, 'softmax')
                    pS = psps.tile([8, 4], F32, name="pS", tag="sps")
                    mm(pS[:], Pg[:], rows[:], start=True, stop=True)
                    rS = wk.tile([8, 4], F32, name="rS")
                    nc.vector.reciprocal(rS[:], pS[:])
                    # rSd cols (g, c): rS[p8, g] * (p8 == c), b-ordered
                    rSd = wk.tile([8, 32], F32, name="rSd")
                    nc.vector.scalar_tensor_tensor(
                        rSd[:].rearrange("p (g c) -> p g c", c=8),
                        rS[:].rearrange("p g -> p g ()").broadcast_to([8, 4, 8]),
                        128.0,
                        Pg[0:8, 0:8].rearrange("p c -> p () c").broadcast_to([8, 4, 8]),
                        ALU.mult, ALU.mult)
                    rbB = psps.tile([128, 32], F32, name="rbB", tag="sps")
                    mm(rbB[:], ones32[:], rSd[:], start=True, stop=True)
                    # HW: DVE may read only ONE non-scalar PSUM input; the
                    # norm stt also reads cps, so rbB must bounce via SBUF
                    rbBs = wk.tile([128, 32], F32, name="rbBs")
                    nc.vector.tensor_copy(rbBs[:], rbB[:])
                    _sec(nc, 'ctx')
                    # normalize while evicting: ctxh = cps * rbBs
                    nc.vector.scalar_tensor_tensor(
                        ctxh[:, w * 128:(w + 1) * 128]
                        .rearrange("p (k c) -> p k c", c=32),
                        cps[:].rearrange("p (k c) -> p k c", c=32), 1.0,
                        rbBs[:].rearrange("p c -> p () c").broadcast_to([128, 4, 32]),
                        ALU.mult, ALU.mult)
                    # -- gates ctx part: bank0 gates (g,f) first, their
                    # activations + c-mul overlap the bank1 (i,o) matmuls
                    # (bank-aligned so no PSUM-bank ping-pong)
                    _sec(nc, 'Wi1+act')
                    gG, gF, gI, gO = (gps[0][:, 0:256], gps[0][:, 256:512],
                                      gps[1][:, 0:256], gps[1][:, 256:512])
                    tGs = wk.tile([128, 256], BF, name="tGs", bufs=2)
                    def wi1_sec(gsec):
                        for bi in range(gsec * 8, gsec * 8 + 8):
                            blk = gblk(gps, bi)
                            for kt in range(4):
                                mm(blk, Wi1[:, kt * 4096 + bi * 128: kt * 4096 + (bi + 1) * 128],
                                   ctxh[:, w * 128 + kt * 32: w * 128 + (kt + 1) * 32],
                                   start=False, stop=(kt == 3))
                    wi1_sec(0)
                    wi1_sec(1)
                    wi1_sec(2)
                    wi1_sec(3)
                    gps_next = gates_iw(t + 1) if t + 1 < T else None
                    _sec(nc, 'Wi1+act')
                    nc.scalar.activation(tGs[:], gG, AF.Tanh, scale=1.0 / (WSCALE * XS))
                    sF = wk.tile([128, 256], BF, name="sF", bufs=2)
                    nc.scalar.activation(sF[:], gF, AF.Sigmoid, scale=1.0 / (WSCALE * XS))
                    nc.vector.tensor_mul(cT[:], cT[:], sF[:])
                    # gate activations land in SBUF bf16: frees the gps PSUM
                    # bank early and enables 2x DVE on the i*g / o*tanh muls
                    gIs = wk.tile([128, 256], BF, name="gIs", bufs=2)
                    nc.scalar.activation(gIs[:], gI, AF.Sigmoid, scale=1.0 / (WSCALE * XS))
                    gOs = wk.tile([128, 256], BF, name="gOs", bufs=2)
                    nc.scalar.activation(gOs[:], gO, AF.Sigmoid, scale=1.0 / (WSCALE * XS))
                    # -- LSTM-gap fillers: next step's dep-free gate matmuls
                    # first (no DMA dependency), then 2 more vocab chunks
                    _sec(nc, 'giw+v2b')
                    # -- p-state heater: dep-free dummy matmuls keep the PE
                    # clock streak alive through the LSTM tail (a broken
                    # streak costs ~3us of half-speed ramp-up)
                    if HEAT_B and t >= 15:
                        heat2 = psm.tile([128, 32], F32, name="heat", tag="x")
                        for _hk in range(HEAT_B):
                            mm(heat2[:], Wa[:, 0:128], Wa[:, 0:32],
                               start=True, stop=True)
                    # -- LSTM elementwise tail (activations emitted above)
                    _sec(nc, 'lstmtail')
                    nc.vector.tensor_mul(gIs[:], gIs[:], tGs[:])
                    nc.vector.tensor_add(cT[:], cT[:], gIs[:])
                    tCs = wk.tile([128, 256], BF, name="tCs", bufs=2)
                    nc.scalar.activation(tCs[:, 0:128], cT[:, 0:128], AF.Tanh)
                    nc.vector.tensor_mul(hist[:, w * 256: w * 256 + 128],
                                         gOs[:, 0:128], tCs[:, 0:128])
                    nc.scalar.activation(tCs[:, 128:256], cT[:, 128:256], AF.Tanh)
                    nc.vector.tensor_mul(hist[:, w * 256 + 128:(w + 1) * 256],
                                         gOs[:, 128:256], tCs[:, 128:256])
                    # e4m3 pair quantization of h for next step's DR gates
                    hsl = hist[:, w * 256:(w + 1) * 256]
                    nc.scalar.activation(hq1[:], hsl, AF.Copy, scale=XS)
                    nc.vector.scalar_tensor_tensor(
                        hq2[:], hsl, XS, hq1[:], ALU.mult, ALU.subtract)
                    # tail-slot vocab emitted AFTER the tail ops: their PE
                    # matmuls still fill this window, but the PSUM->SBUF
                    # copies now queue BEHIND the critical tail ACT/DVE ops
                    # instead of blocking them (in-order engine queues)
                    if t >= 4:
                        vocab_chunks(t // 4 - 1, range(_mid, _hi))
                    gps = gps_next
                # tail: last dec group + its vocab
                dec_group(4)
                vocab_chunks(4, range(16))
                if dbg:
                    nc.sync.dma_start(dbg_hist[:], hist[:])
                    nc.sync.dma_start(dbg_ctxh[:], ctxh[:])
                    nc.sync.dma_start(dbg_cT[:], cT[:])
    nc.finalize()
    return nc


def kernel(**inputs) -> np.ndarray:
    if "nc" not in _BUILT:
        _BUILT["nc"] = build()
    nc = _BUILT["nc"]
    in_maps = host_prep(inputs)
    res = run_bass_kernel_spmd(nc, in_maps, core_ids=list(range(NC)))
    full = np.concatenate(
        [np.asarray(res.results[k]["out"]) for k in range(NC)], axis=1)
    # (640, 32000) bf16, row t*32+b -> (B, T, V) f32
    out = np.ascontiguousarray(
        full.reshape(T, B, V).transpose(1, 0, 2)).astype(np.float32)
    b_out = np.asarray(inputs["b_out"], np.float32)
    if np.any(b_out):
        out += b_out[None, None, :]
    return out

